# revision 4
# baseline (speedup 1.0000x reference)
"""Masked cross-attention (B=4, NQ=100, HW=4096, D=1024, H=16) on 8 TRN2 cores.

Sharding: kv rows (keys) are split 8 ways; each core runs LayerNorm + K/V
projection on its 512-key slice per batch, computes unnormalized partial
attention for all (b, h) against its keys, all-reduces the softmax
denominators on device, normalizes, and computes a partial out-projection.
The host sums the 8 partial outputs and adds the folded bias.

Schedule notes (v2):
 - q + wq load first; Q projection is the first PE work (warms the HAM
   clock gate while kv0/wk stream in), so the first matmul fires ~15us in
   instead of ~40us.
 - ALL kv loads and XBAR transposes are emitted before the first
   collective: any DMA emitted after collective K in program order waits
   for K to complete (one program-order CC counter), which in the old
   schedule stalled kvnT3's transpose ~13us and combine2's transpose
   ~19us.
 - A warm-up AllReduce on a constant fires as soon as the gpsimd queue
   starts, absorbing cross-core launch skew before the first real
   (data-dependent) denominator all-reduce.
 - The mask is applied as an additive -1e9 bias on the DVE directly into
   the scores PSUM before the exp, instead of a post-exp multiply in
   SBUF: fewer SBUF touches next to the streaming matmuls.
 - recip fetches / combine transposes / output DMAs are emitted before
   later all-reduces so only batch 3's combine is gated by the last
   collective.

LayerNorm gamma/beta are folded into the projection weights/biases on the
host; the V-projection bias is folded into the final output bias (exact
because softmax weights sum to one) and the K-projection bias is dropped
entirely (it shifts every key score of a query equally, which softmax
cancels).
"""
import sys

sys.path.insert(0, "/opt/trn_rl_repo")

import numpy as np
import ml_dtypes

import concourse.bacc as bacc
import concourse.bass as bass
import concourse.mybir as mybir
import concourse.tile as tile
from concourse.bass_utils import run_bass_kernel_spmd
B, NQ, HW, D, H = 4, 100, 4096, 1024, 16
HD = D // H          # 64
NCORE = 8
KC = HW // NCORE     # 512 keys per core per batch
NKT = KC // 128      # 4 key sub-tiles of 128
NDC = D // 128       # 8 chunks of the model dim
EPS = 1e-5
SCALE = 1.0 / np.sqrt(np.float32(HD))  # 1/8

F32 = mybir.dt.float32
BF16 = mybir.dt.bfloat16
AF = mybir.ActivationFunctionType
ALU = mybir.AluOpType

_compiled = {}


def _build():
    nc = bacc.Bacc("TRN2", target_bir_lowering=False, num_devices=NCORE)

    kv_d = nc.dram_tensor("kv", [B, NKT, 128, D], BF16, kind="ExternalInput")
    q_d = nc.dram_tensor("q", [B, NQ, D], BF16, kind="ExternalInput")
    # additive mask bias (0 or -1e9), duplicated over the i=2 head-pair dim
    mask_d = nc.dram_tensor("maskT", [128, B, NKT, 2, NQ], BF16,
                            kind="ExternalInput")
    wq_d = nc.dram_tensor("wqT", [128, NDC, D], BF16, kind="ExternalInput")
    wk_d = nc.dram_tensor("wkT", [128, NDC, D], BF16, kind="ExternalInput")
    wv_d = nc.dram_tensor("wvT", [128, NDC, D], BF16, kind="ExternalInput")
    wo_d = nc.dram_tensor("woT", [128, NDC, D], BF16, kind="ExternalInput")
    bq_d = nc.dram_tensor("biasq", [128, NDC], F32, kind="ExternalInput")
    out_d = nc.dram_tensor("out", [128, NDC, B, NQ], BF16, kind="ExternalOutput")

    with tile.TileContext(nc) as tc:
        with (
            tc.tile_pool(name="sb", bufs=1) as sb,
            tc.tile_pool(name="ps", bufs=1, space="PSUM") as ps,
            tc.tile_pool(name="dram", bufs=1, space="DRAM") as dram,
        ):
            # ---- constants ----
            eps_t = sb.tile([128, 1], F32, tag="eps")
            nc.vector.memset(eps_t[:], EPS)
            warm = sb.tile([1, 4], F32, tag="actwarm")
            nc.vector.memset(warm[:], 1.0)
            nc.scalar.activation(warm[:1, 0:1], warm[:1, 2:3], AF.Sqrt)
            nc.scalar.activation(warm[:1, 0:1], warm[:1, 2:3], AF.Copy)
            nc.scalar.activation(warm[:1, 0:1], warm[:1, 2:3], AF.Identity)
            nc.scalar.activation(warm[:1, 0:1], warm[:1, 2:3], AF.Exp)

            wk_sb = sb.tile([128, NDC, D], BF16, tag="wk")
            wv_sb = sb.tile([128, NDC, D], BF16, tag="wv")
            bqv_sb = sb.tile([128, NDC], F32, tag="bqv")
            bq_sb = [bqv_sb[:, j:j + 1] for j in range(NDC)]
            wq_sb = sb.tile([128, NDC, D], BF16, tag="wq", bufs=1, name="wq")
            wo_sb = sb.tile([128, NDC, D], BF16, tag="wo")

            def layernorm_to_bf16(x_bf16, xn_bf16, p):
                """(x - mean) * rsqrt(var + eps), row-wise over the free dim."""
                stats = sb.tile([128, 2, 6], F32, tag="lnstats", bufs=4)
                nc.vector.bn_stats(stats[:p, 0, :], x_bf16[:p, 0:512])
                nc.vector.bn_stats(stats[:p, 1, :], x_bf16[:p, 512:1024])
                mv = sb.tile([128, 2], F32, tag="lnmv", bufs=4)
                nc.vector.bn_aggr(mv[:p], stats[:p])
                rstd = sb.tile([128, 1], F32, tag="lnrstd", bufs=4)
                nc.scalar.activation(rstd[:p], mv[:p, 1:2], AF.Sqrt, bias=eps_t[:p])
                nc.vector.reciprocal(rstd[:p], rstd[:p])
                nc.vector.tensor_scalar(
                    xn_bf16[:p], x_bf16[:p], mv[:p, 0:1], rstd[:p],
                    ALU.subtract, ALU.mult,
                )

            sloc = [dram.tile([NQ, H], F32, tag=f"sloc{b}", name=f"sloc{b}")
                    for b in range(B)]
            sglob = [dram.tile([NQ, H], F32, tag=f"sglob{b}", name=f"sglob{b}")
                     for b in range(B)]
            warm_in = dram.tile([16], F32, tag="warm_in", name="warm_in")
            warm_out = dram.tile([16], F32, tag="warm_out", name="warm_out")
            # normalized, transposed context for all batches: [p, b, k, q]
            ctxT_all = sb.tile([128, B, NDC, 112], BF16, tag="ctxTall")
            NQP = 112  # q rows padded to the XBAR 16-row granule
            qnT = sb.tile([128, B, NDC, NQP], BF16, tag="qnT")
            qpT = []

            def load_kv(b):
                kvraws = []
                for r in range(NKT):
                    kvraw = sb.tile([128, D], BF16, tag="kvraw", bufs=4)
                    nc.sync.dma_start(kvraw[:], kv_d[b, r])
                    kvraws.append(kvraw)
                return kvraws

            def transpose_kv(b, kvraws):
                """LayerNorm + XBAR DMA transpose into kvnT[b].

                kvnT layout: [128 dpart, NKT, NDC, 128 keys]."""
                kvnT = sb.tile([128, NKT, NDC, 128], BF16, tag="kvnT", bufs=2,
                               name=f"kvnT_{b}")
                for r in range(NKT):
                    layernorm_to_bf16(kvraws[r], kvraws[r], 128)
                    nc.scalar.dma_start_transpose(kvnT[:, r], kvraws[r][:])
                return kvnT

            def kproj(b, kvnT):
                """K projection -> kpT[j]: [128 dout, KC keys] (no bias)."""
                kpT = []
                for j in range(NDC):
                    kpT.append(
                        sb.tile([128, KC], BF16, tag=f"kpT{j}", bufs=2,
                                name=f"kpT{j}_{b}")
                    )
                    acc = ps.tile([128, KC], F32, tag="mm", bufs=3)
                    for k in range(NDC):
                        nc.tensor.matmul(
                            acc[:],
                            lhsT=wk_sb[:, k, j * 128:(j + 1) * 128],
                            rhs=kvnT[:, :, k, :],
                            start=(k == 0), stop=(k == NDC - 1),
                        )
                    nc.scalar.activation(kpT[j][:], acc[:], AF.Copy)
                return kpT

            def vproj(b, kvnT):
                """V projection -> vp_ext[r]: [128 keys, H, HD+1], col HD=1."""
                vp_ext = []
                for r in range(NKT):
                    vpe = sb.tile([128, H, HD + 1], BF16, tag=f"vpe{r}", bufs=2,
                                  name=f"vpe{r}_{b}")
                    vp_ext.append(vpe)
                    nc.vector.memset(vpe[:, :, HD:HD + 1], 1.0)
                    for nh in range(2):
                        acc = ps.tile([128, 512], F32, tag="mm", bufs=3)
                        for k in range(NDC):
                            nc.tensor.matmul(
                                acc[:],
                                lhsT=kvnT[:, r, k, :],
                                rhs=wv_sb[:, k, nh * 512:(nh + 1) * 512],
                                start=(k == 0), stop=(k == NDC - 1),
                            )
                        if nh == 0:
                            nc.vector.tensor_copy(
                                out=vpe[:, nh * 8:(nh + 1) * 8, 0:HD],
                                in_=acc[:].rearrange("p (g d) -> p g d", g=8),
                            )
                        else:
                            nc.scalar.activation(
                                vpe[:, nh * 8:(nh + 1) * 8, 0:HD],
                                acc[:].rearrange("p (g d) -> p g d", g=8),
                                AF.Copy,
                            )
                return vp_ext

            def load_q():
                qraws = []
                for b in range(B):
                    qraw = sb.tile([NQ, D], BF16, tag="qraw", bufs=2)
                    nc.sync.dma_start(qraw[:], q_d[b])
                    qraws.append(qraw)
                return qraws

            def ln_T_q(qraws):
                for b in range(B):
                    qn = sb.tile([NQP, D], BF16, tag="qn", bufs=1)
                    layernorm_to_bf16(qraws[b], qn, NQ)
                    nc.scalar.dma_start_transpose(qnT[:, b], qn[:])

            def qproj():
                """qpT_pad[j]: [128, B, 2, NQ] block-diagonal by head."""
                for j in range(NDC):
                    qpT.append(
                        sb.tile([128, B, 2, NQ], BF16, tag=f"qpT{j}",
                                name=f"qpT{j}")
                    )
                    nc.gpsimd.memset(qpT[j][:], 0.0)
                    acc = ps.tile([128, B * NQ], F32, tag="sc", bufs=2)
                    for k in range(NDC):
                        nc.tensor.matmul(
                            acc[:],
                            lhsT=wq_sb[:, k, j * 128:(j + 1) * 128],
                            rhs=qnT[:, :, k, 0:NQ],
                            start=(k == 0), stop=(k == NDC - 1),
                        )
                    nc.scalar.activation(
                        qpT[j][0:HD, :, 0, :],
                        acc[0:HD, :].rearrange("p (b q) -> p b q", b=B),
                        AF.Identity, bias=bq_sb[j][0:HD],
                    )
                    nc.scalar.activation(
                        qpT[j][HD:128, :, 1, :],
                        acc[HD:128, :].rearrange("p (b q) -> p b q", b=B),
                        AF.Identity, bias=bq_sb[j][HD:128],
                    )

            mask_all = sb.tile([128, B, NKT, 2, NQ], BF16, tag="maskb")

            def scores_exp(b, kpT):
                """scores^T + additive mask bias + exp for all heads of b."""
                exp_all = sb.tile([128, NKT, H, NQ], BF16, tag="expall", bufs=2,
                                  name=f"exp_{b}")
                for j in range(NDC):
                    for c2 in range(2):
                        sc = ps.tile([128, 2, 2, NQ], F32, tag="sc", bufs=2)
                        for cc in range(2):
                            c = 2 * c2 + cc
                            nc.tensor.matmul(
                                sc[:, cc, :, :].rearrange("p i q -> p (i q)"),
                                lhsT=kpT[j][:, c * 128:(c + 1) * 128],
                                rhs=qpT[j][:, b, :, :].rearrange(
                                    "p i q -> p (i q)"),
                                start=True, stop=True,
                            )
                        for cc in range(2):
                            c = 2 * c2 + cc
                            nc.vector.tensor_add(
                                sc[:, cc, :, :], sc[:, cc, :, :],
                                mask_all[:, b, c, :, :],
                            )
                        nc.scalar.activation(
                            exp_all[:, 2 * c2:2 * c2 + 2, 2 * j:2 * j + 2, :],
                            sc[:], AF.Exp, scale=float(SCALE),
                        )
                return exp_all

            def ctx_block(b, exp_all, vp_ext):
                """Unnormalized ctx + denominators for batch b; DMA the local
                denominators out for the all-reduce."""
                ctx_b = sb.tile([NQ, H, HD + 1], F32, tag="ctxsb", bufs=4,
                                name=f"ctx_{b}")
                for h in range(H):
                    ctx_ps = ps.tile([NQ, HD + 1], F32, tag="small", bufs=3)
                    for c in range(NKT):
                        nc.tensor.matmul(
                            ctx_ps[:],
                            lhsT=exp_all[:, c, h, :],
                            rhs=vp_ext[c][:, h, :],
                            start=(c == 0), stop=(c == NKT - 1),
                        )
                    if h % 2 == 0:
                        nc.vector.tensor_copy(out=ctx_b[:, h, :], in_=ctx_ps[:])
                    else:
                        nc.scalar.activation(ctx_b[:, h, :], ctx_ps[:], AF.Copy)
                den = sb.tile([NQ, H], F32, tag="den", bufs=4)
                nc.gpsimd.tensor_copy(out=den[:], in_=ctx_b[:, :, HD])
                nc.sync.dma_start(sloc[b][:], den[:])
                return ctx_b

            def allreduce_b(b):
                nc.gpsimd.collective_compute(
                    "AllReduce", ALU.add,
                    replica_groups=[list(range(NCORE))],
                    ins=[sloc[b][:].opt()], outs=[sglob[b][:].opt()],
                )
                return b

            def recip_fetch(b):
                recip = sb.tile([NQ, H], F32, tag="recip", bufs=4,
                                name=f"recip_{b}")
                nc.scalar.dma_start(recip[:], sglob[b][:])
                return recip

            def combine_block(b, ctx_b, recip):
                """Normalize by global denominators and XBAR-transpose into
                ctxT_all."""
                nc.vector.reciprocal(recip[:], recip[:])
                ctxn = sb.tile([112, H, HD], BF16, tag="ctxn", bufs=1)
                for h in range(H):
                    if h % 2 == 0:
                        nc.vector.tensor_scalar_mul(
                            ctxn[:NQ, h, :], ctx_b[:, h, 0:HD],
                            recip[:, h:h + 1]
                        )
                    else:
                        nc.scalar.activation(
                            ctxn[:NQ, h, :], ctx_b[:, h, 0:HD], AF.Copy,
                            scale=recip[:, h:h + 1],
                        )
                nc.scalar.dma_start_transpose(ctxT_all[:, b], ctxn[:])

            # reuses wq's slot (wq is dead after the Q projection)
            out_sb = sb.tile([128, NDC, B, NQ], BF16, tag="wq", bufs=1,
                             name="out_sb")

            def outproj(b0, nb):
                for m in range(NDC):
                    acc = ps.tile([128, nb, NQ], F32, tag="sc", bufs=2)
                    for k in range(NDC):
                        nc.tensor.matmul(
                            acc[:],
                            lhsT=wo_sb[:, k, m * 128:(m + 1) * 128],
                            rhs=ctxT_all[:, b0:b0 + nb, k, 0:NQ],
                            start=(k == 0), stop=(k == NDC - 1),
                        )
                    if m % 2 == 0:
                        nc.vector.tensor_copy(
                            out=out_sb[:, m, b0:b0 + nb, :], in_=acc[:]
                        )
                    else:
                        nc.scalar.activation(
                            out_sb[:, m, b0:b0 + nb, :], acc[:], AF.Copy,
                        )
                nc.sync.dma_start(
                    out_d[:, :, b0:b0 + nb, :], out_sb[:, :, b0:b0 + nb, :]
                )

            # ---- pipelined schedule (v2) ----
            # DMA queue: q, bq, wq first (Q path gates nothing else and PE
            # starts on qproj while kv0/wk stream).  All kv loads and all
            # XBAR transposes are emitted before the first collective.
            qraws = load_q()
            nc.sync.dma_start(bqv_sb[:], bq_d[:])
            nc.sync.dma_start(wq_sb[:], wq_d[:])
            kvraws0 = load_kv(0)
            nc.sync.dma_start(wk_sb[:], wk_d[:])

            ln_T_q(qraws)
            qproj()

            kvnT0 = transpose_kv(0, kvraws0)
            kvraws1 = load_kv(1)
            nc.sync.dma_start(wv_sb[:], wv_d[:])
            nc.sync.dma_start(mask_all[:], mask_d[:])

            kpT0 = kproj(0, kvnT0)
            kvnT1 = transpose_kv(1, kvraws1)
            kvraws2 = load_kv(2)
            nc.sync.dma_start(wo_sb[:], wo_d[:])
            vp0 = vproj(0, kvnT0)
            kvnT2 = transpose_kv(2, kvraws2)
            kvraws3 = load_kv(3)
            kvnT3 = transpose_kv(3, kvraws3)

            # skew-absorbing warm-up collective: no data deps, fires as soon
            # as the gpsimd queue reaches it; completes long before any DMA
            # emitted after it becomes ready to run.
            nc.gpsimd.collective_compute(
                "AllReduce", ALU.add,
                replica_groups=[list(range(NCORE))],
                ins=[warm_in[:].opt()], outs=[warm_out[:].opt()],
            )

            exp0 = scores_exp(0, kpT0)
            kpT1 = kproj(1, kvnT1)
            ctx0 = ctx_block(0, exp0, vp0)
            allreduce_b(0)
            vp1 = vproj(1, kvnT1)

            exp1 = scores_exp(1, kpT1)
            kpT2 = kproj(2, kvnT2)
            ctx1 = ctx_block(1, exp1, vp1)
            allreduce_b(1)
            r0 = recip_fetch(0)
            vp2 = vproj(2, kvnT2)

            exp2 = scores_exp(2, kpT2)
            kpT3 = kproj(3, kvnT3)
            ctx2 = ctx_block(2, exp2, vp2)
            allreduce_b(2)
            r1 = recip_fetch(1)
            vp3 = vproj(3, kvnT3)

            exp3 = scores_exp(3, kpT3)
            ctx3 = ctx_block(3, exp3, vp3)
            r2 = recip_fetch(2)
            combine_block(0, ctx0, r0)
            combine_block(1, ctx1, r1)
            outproj(0, 2)
            combine_block(2, ctx2, r2)
            outproj(2, 1)
            allreduce_b(3)
            r3 = recip_fetch(3)
            combine_block(3, ctx3, r3)
            outproj(3, 1)

    nc.compile()
    return nc


def _prep_in_maps(q, kv, mask, in_proj_w, in_proj_b, out_w, out_b,
                  g_q, b_q, g_kv, b_kv):
    """Host-side prep: fold LN affine + V-bias, shard kv/mask per core.

    Returns (in_maps, bias_total)."""
    q = np.asarray(q, np.float32)
    kv = np.asarray(kv, np.float32)
    mask = np.asarray(mask)
    in_proj_w = np.asarray(in_proj_w, np.float32)
    in_proj_b = np.asarray(in_proj_b, np.float32)
    out_w = np.asarray(out_w, np.float32)
    out_b = np.asarray(out_b, np.float32)
    g_q = np.asarray(g_q, np.float32)
    b_q = np.asarray(b_q, np.float32)
    g_kv = np.asarray(g_kv, np.float32)
    b_kv = np.asarray(b_kv, np.float32)

    Wq, Wk, Wv = in_proj_w[:D], in_proj_w[D:2 * D], in_proj_w[2 * D:]
    bq, bk, bv = in_proj_b[:D], in_proj_b[D:2 * D], in_proj_b[2 * D:]

    # Fold LayerNorm affine into projections: LN(x)*g+b @ W^T + c
    #   = LN(x) @ (W*g)^T + (W@b + c)
    WqT = (Wq * g_q[None, :]).T.astype(ml_dtypes.bfloat16)
    WkT = (Wk * g_kv[None, :]).T.astype(ml_dtypes.bfloat16)
    WvT = (Wv * g_kv[None, :]).T.astype(ml_dtypes.bfloat16)
    bq_eff = (bq + Wq @ b_q).astype(np.float32)
    bv_eff = (bv + Wv @ b_kv).astype(np.float32)
    # K bias cancels in softmax; V bias folds into the output bias.
    WoT = out_w.T.astype(ml_dtypes.bfloat16)
    bias_total = (out_b + out_w @ bv_eff).astype(np.float32)

    # per-query key mask; all-zero mask rows attend everywhere
    kv16 = kv.astype(ml_dtypes.bfloat16)
    allowed = (mask != 0)
    has_any = allowed.any(axis=-1, keepdims=True)
    eff = np.where(has_any, allowed, True)  # [B, NQ, HW] bool

    common = {
        "q": np.ascontiguousarray(q.astype(ml_dtypes.bfloat16)),
        "wqT": np.ascontiguousarray(WqT.reshape(NDC, 128, D).transpose(1, 0, 2)),
        "wkT": np.ascontiguousarray(WkT.reshape(NDC, 128, D).transpose(1, 0, 2)),
        "wvT": np.ascontiguousarray(WvT.reshape(NDC, 128, D).transpose(1, 0, 2)),
        "woT": np.ascontiguousarray(WoT.reshape(NDC, 128, D).transpose(1, 0, 2)),
        "biasq": np.ascontiguousarray(bq_eff.reshape(NDC, 128).T),
    }
    in_maps = []
    for c in range(NCORE):
        sl = slice(c * KC, (c + 1) * KC)
        kv_c = kv16[:, sl, :].reshape(B, NKT, 128, D)
        # additive mask bias slice -> [128, B, NKT, 2, NQ] bf16
        m_c = eff[:, :, sl].transpose(0, 2, 1).reshape(B, NKT, 128, NQ)
        mb = np.where(m_c, np.float32(0.0), np.float32(-1e9))
        mb = mb.transpose(2, 0, 1, 3)  # [128, B, NKT, NQ]
        mb = np.broadcast_to(mb[:, :, :, None, :], (128, B, NKT, 2, NQ))
        in_maps.append({
            **common,
            "kv": np.ascontiguousarray(kv_c),
            "maskT": np.ascontiguousarray(mb.astype(ml_dtypes.bfloat16)),
        })
    return in_maps, bias_total


def kernel(q, kv, mask, in_proj_w, in_proj_b, out_w, out_b, g_q, b_q, g_kv, b_kv):
    in_maps, bias_total = _prep_in_maps(
        q, kv, mask, in_proj_w, in_proj_b, out_w, out_b, g_q, b_q, g_kv, b_kv
    )
    if "nc" not in _compiled:
        _compiled["nc"] = _build()
    nc = _compiled["nc"]

    res = run_bass_kernel_spmd(nc, in_maps, core_ids=list(range(NCORE)))

    out = np.zeros((B, NQ, D), np.float32)
    for c in range(NCORE):
        part = res.results[c]["out"]  # [128 p, NDC m, B, NQ]; dout = m*128+p
        out += part.transpose(2, 3, 1, 0).reshape(B, NQ, D).astype(np.float32)
    out += bias_total[None, None, :]
    return out


# revision 6
# speedup vs baseline: 1.0645x; 1.0645x over previous
"""Masked cross-attention (B=4, NQ=100, HW=4096, D=1024, H=16) on 8 TRN2 cores.

Sharding: kv rows (keys) are split 8 ways; each core runs LayerNorm + K/V
projection on its 512-key slice per batch, computes unnormalized partial
attention for all (b, h) against its keys, all-reduces the softmax
denominators on device, normalizes, and computes a partial out-projection.
The host sums the 8 partial outputs and adds the folded bias.

Schedule notes (v2):
 - q + wq load first; Q projection is the first PE work (warms the HAM
   clock gate while kv0/wk stream in), so the first matmul fires ~15us in
   instead of ~40us.
 - ALL kv loads and XBAR transposes are emitted before the first
   collective: any DMA emitted after collective K in program order waits
   for K to complete (one program-order CC counter), which in the old
   schedule stalled kvnT3's transpose ~13us and combine2's transpose
   ~19us.
 - A warm-up AllReduce on a constant fires as soon as the gpsimd queue
   starts, absorbing cross-core launch skew before the first real
   (data-dependent) denominator all-reduce.
 - The mask is applied as an additive -1e9 bias on the DVE directly into
   the scores PSUM before the exp, instead of a post-exp multiply in
   SBUF: fewer SBUF touches next to the streaming matmuls.
 - recip fetches / combine transposes / output DMAs are emitted before
   later all-reduces so only batch 3's combine is gated by the last
   collective.

LayerNorm gamma/beta are folded into the projection weights/biases on the
host; the V-projection bias is folded into the final output bias (exact
because softmax weights sum to one) and the K-projection bias is dropped
entirely (it shifts every key score of a query equally, which softmax
cancels).
"""
import sys

sys.path.insert(0, "/opt/trn_rl_repo")

import numpy as np
import ml_dtypes

import concourse.bacc as bacc
import concourse.bass as bass
import concourse.mybir as mybir
import concourse.tile as tile
from concourse.bass_utils import run_bass_kernel_spmd
B, NQ, HW, D, H = 4, 100, 4096, 1024, 16
HD = D // H          # 64
NCORE = 8
KC = HW // NCORE     # 512 keys per core per batch
NKT = KC // 128      # 4 key sub-tiles of 128
NDC = D // 128       # 8 chunks of the model dim
EPS = 1e-5
SCALE = 1.0 / np.sqrt(np.float32(HD))  # 1/8

F32 = mybir.dt.float32
BF16 = mybir.dt.bfloat16
AF = mybir.ActivationFunctionType
ALU = mybir.AluOpType

_compiled = {}


def _build():
    nc = bacc.Bacc("TRN2", target_bir_lowering=False, num_devices=NCORE)

    kv_d = nc.dram_tensor("kv", [B, NKT, 128, D], BF16, kind="ExternalInput")
    q_d = nc.dram_tensor("q", [B, NQ, D], BF16, kind="ExternalInput")
    # additive mask bias (0 or -1e9), duplicated over the i=2 head-pair dim
    mask_d = nc.dram_tensor("maskT", [128, B, NKT, 2, NQ], BF16,
                            kind="ExternalInput")
    wq_d = nc.dram_tensor("wqT", [128, NDC, D], BF16, kind="ExternalInput")
    wk_d = nc.dram_tensor("wkT", [128, NDC, D], BF16, kind="ExternalInput")
    wv_d = nc.dram_tensor("wvT", [128, NDC, D], BF16, kind="ExternalInput")
    wo_d = nc.dram_tensor("woT", [128, NDC, D], BF16, kind="ExternalInput")
    bq_d = nc.dram_tensor("biasq", [128, NDC], F32, kind="ExternalInput")
    out_d = nc.dram_tensor("out", [128, NDC, B, NQ], BF16, kind="ExternalOutput")

    with tile.TileContext(nc) as tc:
        with (
            tc.tile_pool(name="sb", bufs=1) as sb,
            tc.tile_pool(name="ps", bufs=1, space="PSUM") as ps,
            tc.tile_pool(name="dram", bufs=1, space="DRAM") as dram,
        ):
            # ---- constants ----
            eps_t = sb.tile([128, 1], F32, tag="eps")
            nc.vector.memset(eps_t[:], EPS)
            warm = sb.tile([1, 4], F32, tag="actwarm")
            nc.vector.memset(warm[:], 1.0)
            nc.scalar.activation(warm[:1, 0:1], warm[:1, 2:3], AF.Sqrt)
            nc.scalar.activation(warm[:1, 0:1], warm[:1, 2:3], AF.Copy)
            nc.scalar.activation(warm[:1, 0:1], warm[:1, 2:3], AF.Identity)
            nc.scalar.activation(warm[:1, 0:1], warm[:1, 2:3], AF.Exp)

            wk_sb = sb.tile([128, NDC, D], BF16, tag="wk")
            wv_sb = sb.tile([128, NDC, D], BF16, tag="wv")
            bqv_sb = sb.tile([128, NDC], F32, tag="bqv")
            bq_sb = [bqv_sb[:, j:j + 1] for j in range(NDC)]
            wq_sb = sb.tile([128, NDC, D], BF16, tag="wq", bufs=1, name="wq")
            wo_sb = sb.tile([128, NDC, D], BF16, tag="wo")

            def layernorm_to_bf16(x_bf16, xn_bf16, p):
                """(x - mean) * rsqrt(var + eps), row-wise over the free dim."""
                stats = sb.tile([128, 2, 6], F32, tag="lnstats", bufs=4)
                nc.vector.bn_stats(stats[:p, 0, :], x_bf16[:p, 0:512])
                nc.vector.bn_stats(stats[:p, 1, :], x_bf16[:p, 512:1024])
                mv = sb.tile([128, 2], F32, tag="lnmv", bufs=4)
                nc.vector.bn_aggr(mv[:p], stats[:p])
                rstd = sb.tile([128, 1], F32, tag="lnrstd", bufs=4)
                nc.scalar.activation(rstd[:p], mv[:p, 1:2], AF.Sqrt, bias=eps_t[:p])
                nc.vector.reciprocal(rstd[:p], rstd[:p])
                nc.vector.tensor_scalar(
                    xn_bf16[:p], x_bf16[:p], mv[:p, 0:1], rstd[:p],
                    ALU.subtract, ALU.mult,
                )

            sloc = [dram.tile([NQ, H], F32, tag=f"sloc{b}", name=f"sloc{b}")
                    for b in range(B)]
            sglob = [dram.tile([NQ, H], F32, tag=f"sglob{b}", name=f"sglob{b}")
                     for b in range(B)]
            # normalized, transposed context for all batches: [p, b, k, q]
            ctxT_all = sb.tile([128, B, NDC, 112], BF16, tag="ctxTall")
            NQP = 112  # q rows padded to the XBAR 16-row granule
            qnT = sb.tile([128, B, NDC, NQP], BF16, tag="qnT")
            qpT = []

            def load_kv(b):
                kvraws = []
                for r in range(NKT):
                    kvraw = sb.tile([128, D], BF16, tag="kvraw", bufs=4)
                    nc.sync.dma_start(kvraw[:], kv_d[b, r])
                    kvraws.append(kvraw)
                return kvraws

            def transpose_kv(b, kvraws):
                """LayerNorm + XBAR DMA transpose into kvnT[b].

                kvnT layout: [128 dpart, NKT, NDC, 128 keys]."""
                kvnT = sb.tile([128, NKT, NDC, 128], BF16, tag="kvnT", bufs=2,
                               name=f"kvnT_{b}")
                for r in range(NKT):
                    layernorm_to_bf16(kvraws[r], kvraws[r], 128)
                    nc.scalar.dma_start_transpose(kvnT[:, r], kvraws[r][:])
                return kvnT

            def kproj(b, kvnT):
                """K projection -> kpT[j]: [128 dout, KC keys] (no bias)."""
                kpT = []
                for j in range(NDC):
                    kpT.append(
                        sb.tile([128, KC], BF16, tag=f"kpT{j}", bufs=2,
                                name=f"kpT{j}_{b}")
                    )
                    acc = ps.tile([128, KC], F32, tag="mm", bufs=3)
                    for k in range(NDC):
                        nc.tensor.matmul(
                            acc[:],
                            lhsT=wk_sb[:, k, j * 128:(j + 1) * 128],
                            rhs=kvnT[:, :, k, :],
                            start=(k == 0), stop=(k == NDC - 1),
                        )
                    nc.scalar.activation(kpT[j][:], acc[:], AF.Copy)
                return kpT

            def vproj(b, kvnT):
                """V projection -> vp_ext[r]: [128 keys, H, HD+1], col HD=1."""
                vp_ext = []
                for r in range(NKT):
                    vpe = sb.tile([128, H, HD + 1], BF16, tag=f"vpe{r}", bufs=2,
                                  name=f"vpe{r}_{b}")
                    vp_ext.append(vpe)
                    nc.vector.memset(vpe[:, :, HD:HD + 1], 1.0)
                    for nh in range(2):
                        acc = ps.tile([128, 512], F32, tag="mm", bufs=3)
                        for k in range(NDC):
                            nc.tensor.matmul(
                                acc[:],
                                lhsT=kvnT[:, r, k, :],
                                rhs=wv_sb[:, k, nh * 512:(nh + 1) * 512],
                                start=(k == 0), stop=(k == NDC - 1),
                            )
                        if nh == 0:
                            nc.vector.tensor_copy(
                                out=vpe[:, nh * 8:(nh + 1) * 8, 0:HD],
                                in_=acc[:].rearrange("p (g d) -> p g d", g=8),
                            )
                        else:
                            nc.scalar.activation(
                                vpe[:, nh * 8:(nh + 1) * 8, 0:HD],
                                acc[:].rearrange("p (g d) -> p g d", g=8),
                                AF.Copy,
                            )
                return vp_ext

            def load_q():
                qraws = []
                for b in range(B):
                    qraw = sb.tile([NQ, D], BF16, tag="qraw", bufs=2)
                    nc.sync.dma_start(qraw[:], q_d[b])
                    qraws.append(qraw)
                return qraws

            def ln_T_q(qraws):
                for b in range(B):
                    qn = sb.tile([NQP, D], BF16, tag="qn", bufs=1)
                    layernorm_to_bf16(qraws[b], qn, NQ)
                    nc.scalar.dma_start_transpose(qnT[:, b], qn[:])

            def qproj():
                """qpT_pad[j]: [128, B, 2, NQ] block-diagonal by head."""
                for j in range(NDC):
                    qpT.append(
                        sb.tile([128, B, 2, NQ], BF16, tag=f"qpT{j}",
                                name=f"qpT{j}")
                    )
                    nc.gpsimd.memset(qpT[j][:], 0.0)
                    acc = ps.tile([128, B * NQ], F32, tag="sc", bufs=2)
                    for k in range(NDC):
                        nc.tensor.matmul(
                            acc[:],
                            lhsT=wq_sb[:, k, j * 128:(j + 1) * 128],
                            rhs=qnT[:, :, k, 0:NQ],
                            start=(k == 0), stop=(k == NDC - 1),
                        )
                    nc.scalar.activation(
                        qpT[j][0:HD, :, 0, :],
                        acc[0:HD, :].rearrange("p (b q) -> p b q", b=B),
                        AF.Identity, bias=bq_sb[j][0:HD],
                    )
                    nc.scalar.activation(
                        qpT[j][HD:128, :, 1, :],
                        acc[HD:128, :].rearrange("p (b q) -> p b q", b=B),
                        AF.Identity, bias=bq_sb[j][HD:128],
                    )

            mask_all = sb.tile([128, B, NKT, 2, NQ], BF16, tag="maskb")

            def scores_exp(b, kpT):
                """scores^T + additive mask bias + exp for all heads of b."""
                exp_all = sb.tile([128, NKT, H, NQ], BF16, tag="expall", bufs=2,
                                  name=f"exp_{b}")
                for j in range(NDC):
                    for c2 in range(2):
                        sc = ps.tile([128, 2, 2, NQ], F32, tag="sc", bufs=2)
                        for cc in range(2):
                            c = 2 * c2 + cc
                            nc.tensor.matmul(
                                sc[:, cc, :, :].rearrange("p i q -> p (i q)"),
                                lhsT=kpT[j][:, c * 128:(c + 1) * 128],
                                rhs=qpT[j][:, b, :, :].rearrange(
                                    "p i q -> p (i q)"),
                                start=True, stop=True,
                            )
                        for cc in range(2):
                            c = 2 * c2 + cc
                            nc.vector.tensor_add(
                                sc[:, cc, :, :], sc[:, cc, :, :],
                                mask_all[:, b, c, :, :],
                            )
                        nc.scalar.activation(
                            exp_all[:, 2 * c2:2 * c2 + 2, 2 * j:2 * j + 2, :],
                            sc[:], AF.Exp, scale=float(SCALE),
                        )
                return exp_all

            def ctx_block(b, exp_all, vp_ext):
                """Unnormalized ctx + denominators for batch b; DMA the local
                denominators out for the all-reduce."""
                ctx_b = sb.tile([NQ, H, HD + 1], F32, tag="ctxsb", bufs=4,
                                name=f"ctx_{b}")
                for h in range(H):
                    ctx_ps = ps.tile([NQ, HD + 1], F32, tag="small", bufs=3)
                    for c in range(NKT):
                        nc.tensor.matmul(
                            ctx_ps[:],
                            lhsT=exp_all[:, c, h, :],
                            rhs=vp_ext[c][:, h, :],
                            start=(c == 0), stop=(c == NKT - 1),
                        )
                    if h % 2 == 0:
                        nc.vector.tensor_copy(out=ctx_b[:, h, :], in_=ctx_ps[:])
                    else:
                        nc.scalar.activation(ctx_b[:, h, :], ctx_ps[:], AF.Copy)
                den = sb.tile([NQ, H], F32, tag="den", bufs=4)
                nc.gpsimd.tensor_copy(out=den[:], in_=ctx_b[:, :, HD])
                nc.sync.dma_start(sloc[b][:], den[:])
                return ctx_b

            def allreduce_b(b):
                nc.gpsimd.collective_compute(
                    "AllReduce", ALU.add,
                    replica_groups=[list(range(NCORE))],
                    ins=[sloc[b][:].opt()], outs=[sglob[b][:].opt()],
                )
                return b

            def recip_fetch(b):
                recip = sb.tile([NQ, H], F32, tag="recip", bufs=4,
                                name=f"recip_{b}")
                nc.scalar.dma_start(recip[:], sglob[b][:])
                return recip

            def combine_block(b, ctx_b, recip):
                """Normalize by global denominators and XBAR-transpose into
                ctxT_all."""
                nc.vector.reciprocal(recip[:], recip[:])
                ctxn = sb.tile([112, H, HD], BF16, tag="ctxn", bufs=1)
                for h in range(H):
                    if h % 2 == 0:
                        nc.vector.tensor_scalar_mul(
                            ctxn[:NQ, h, :], ctx_b[:, h, 0:HD],
                            recip[:, h:h + 1]
                        )
                    else:
                        nc.scalar.activation(
                            ctxn[:NQ, h, :], ctx_b[:, h, 0:HD], AF.Copy,
                            scale=recip[:, h:h + 1],
                        )
                nc.scalar.dma_start_transpose(ctxT_all[:, b], ctxn[:])

            # reuses wq's slot (wq is dead after the Q projection)
            out_sb = sb.tile([128, NDC, B, NQ], BF16, tag="wq", bufs=1,
                             name="out_sb")

            def outproj(b0, nb):
                for m in range(NDC):
                    acc = ps.tile([128, nb, NQ], F32, tag="sc", bufs=2)
                    for k in range(NDC):
                        nc.tensor.matmul(
                            acc[:],
                            lhsT=wo_sb[:, k, m * 128:(m + 1) * 128],
                            rhs=ctxT_all[:, b0:b0 + nb, k, 0:NQ],
                            start=(k == 0), stop=(k == NDC - 1),
                        )
                    if m % 2 == 0:
                        nc.vector.tensor_copy(
                            out=out_sb[:, m, b0:b0 + nb, :], in_=acc[:]
                        )
                    else:
                        nc.scalar.activation(
                            out_sb[:, m, b0:b0 + nb, :], acc[:], AF.Copy,
                        )
                nc.sync.dma_start(
                    out_d[:, :, b0:b0 + nb, :], out_sb[:, :, b0:b0 + nb, :]
                )

            # ---- pipelined schedule (v2) ----
            # DMA queue: q, bq, wq first (Q path gates nothing else and PE
            # starts on qproj while kv0/wk stream).  All kv loads and all
            # XBAR transposes are emitted before the first collective.
            qraws = load_q()
            nc.sync.dma_start(bqv_sb[:], bq_d[:])
            nc.sync.dma_start(wq_sb[:], wq_d[:])
            kvraws0 = load_kv(0)
            nc.sync.dma_start(wk_sb[:], wk_d[:])

            ln_T_q(qraws)
            qproj()

            kvnT0 = transpose_kv(0, kvraws0)
            kvraws1 = load_kv(1)
            nc.sync.dma_start(wv_sb[:], wv_d[:])
            nc.sync.dma_start(mask_all[:], mask_d[:])

            kpT0 = kproj(0, kvnT0)
            kvnT1 = transpose_kv(1, kvraws1)
            kvraws2 = load_kv(2)
            nc.sync.dma_start(wo_sb[:], wo_d[:])
            vp0 = vproj(0, kvnT0)
            kvnT2 = transpose_kv(2, kvraws2)
            kvraws3 = load_kv(3)
            kvnT3 = transpose_kv(3, kvraws3)

            exp0 = scores_exp(0, kpT0)
            kpT1 = kproj(1, kvnT1)
            ctx0 = ctx_block(0, exp0, vp0)
            allreduce_b(0)
            r0 = recip_fetch(0)
            combine_block(0, ctx0, r0)
            vp1 = vproj(1, kvnT1)

            exp1 = scores_exp(1, kpT1)
            kpT2 = kproj(2, kvnT2)
            ctx1 = ctx_block(1, exp1, vp1)
            allreduce_b(1)
            r1 = recip_fetch(1)
            combine_block(1, ctx1, r1)
            outproj(0, 2)
            vp2 = vproj(2, kvnT2)

            exp2 = scores_exp(2, kpT2)
            kpT3 = kproj(3, kvnT3)
            ctx2 = ctx_block(2, exp2, vp2)
            allreduce_b(2)
            r2 = recip_fetch(2)
            combine_block(2, ctx2, r2)
            outproj(2, 1)
            vp3 = vproj(3, kvnT3)

            exp3 = scores_exp(3, kpT3)
            ctx3 = ctx_block(3, exp3, vp3)
            allreduce_b(3)
            r3 = recip_fetch(3)
            combine_block(3, ctx3, r3)
            outproj(3, 1)

    nc.compile()
    return nc


def _prep_in_maps(q, kv, mask, in_proj_w, in_proj_b, out_w, out_b,
                  g_q, b_q, g_kv, b_kv):
    """Host-side prep: fold LN affine + V-bias, shard kv/mask per core.

    Returns (in_maps, bias_total)."""
    q = np.asarray(q, np.float32)
    kv = np.asarray(kv, np.float32)
    mask = np.asarray(mask)
    in_proj_w = np.asarray(in_proj_w, np.float32)
    in_proj_b = np.asarray(in_proj_b, np.float32)
    out_w = np.asarray(out_w, np.float32)
    out_b = np.asarray(out_b, np.float32)
    g_q = np.asarray(g_q, np.float32)
    b_q = np.asarray(b_q, np.float32)
    g_kv = np.asarray(g_kv, np.float32)
    b_kv = np.asarray(b_kv, np.float32)

    Wq, Wk, Wv = in_proj_w[:D], in_proj_w[D:2 * D], in_proj_w[2 * D:]
    bq, bk, bv = in_proj_b[:D], in_proj_b[D:2 * D], in_proj_b[2 * D:]

    # Fold LayerNorm affine into projections: LN(x)*g+b @ W^T + c
    #   = LN(x) @ (W*g)^T + (W@b + c)
    WqT = (Wq * g_q[None, :]).T.astype(ml_dtypes.bfloat16)
    WkT = (Wk * g_kv[None, :]).T.astype(ml_dtypes.bfloat16)
    WvT = (Wv * g_kv[None, :]).T.astype(ml_dtypes.bfloat16)
    bq_eff = (bq + Wq @ b_q).astype(np.float32)
    bv_eff = (bv + Wv @ b_kv).astype(np.float32)
    # K bias cancels in softmax; V bias folds into the output bias.
    WoT = out_w.T.astype(ml_dtypes.bfloat16)
    bias_total = (out_b + out_w @ bv_eff).astype(np.float32)

    # per-query key mask; all-zero mask rows attend everywhere
    kv16 = kv.astype(ml_dtypes.bfloat16)
    allowed = (mask != 0)
    has_any = allowed.any(axis=-1, keepdims=True)
    eff = np.where(has_any, allowed, True)  # [B, NQ, HW] bool

    common = {
        "q": np.ascontiguousarray(q.astype(ml_dtypes.bfloat16)),
        "wqT": np.ascontiguousarray(WqT.reshape(NDC, 128, D).transpose(1, 0, 2)),
        "wkT": np.ascontiguousarray(WkT.reshape(NDC, 128, D).transpose(1, 0, 2)),
        "wvT": np.ascontiguousarray(WvT.reshape(NDC, 128, D).transpose(1, 0, 2)),
        "woT": np.ascontiguousarray(WoT.reshape(NDC, 128, D).transpose(1, 0, 2)),
        "biasq": np.ascontiguousarray(bq_eff.reshape(NDC, 128).T),
    }
    in_maps = []
    for c in range(NCORE):
        sl = slice(c * KC, (c + 1) * KC)
        kv_c = kv16[:, sl, :].reshape(B, NKT, 128, D)
        # additive mask bias slice -> [128, B, NKT, 2, NQ] bf16
        m_c = eff[:, :, sl].transpose(0, 2, 1).reshape(B, NKT, 128, NQ)
        mb = np.where(m_c, np.float32(0.0), np.float32(-1e9))
        mb = mb.transpose(2, 0, 1, 3)  # [128, B, NKT, NQ]
        mb = np.broadcast_to(mb[:, :, :, None, :], (128, B, NKT, 2, NQ))
        in_maps.append({
            **common,
            "kv": np.ascontiguousarray(kv_c),
            "maskT": np.ascontiguousarray(mb.astype(ml_dtypes.bfloat16)),
        })
    return in_maps, bias_total


def kernel(q, kv, mask, in_proj_w, in_proj_b, out_w, out_b, g_q, b_q, g_kv, b_kv):
    in_maps, bias_total = _prep_in_maps(
        q, kv, mask, in_proj_w, in_proj_b, out_w, out_b, g_q, b_q, g_kv, b_kv
    )
    if "nc" not in _compiled:
        _compiled["nc"] = _build()
    nc = _compiled["nc"]

    res = run_bass_kernel_spmd(nc, in_maps, core_ids=list(range(NCORE)))

    out = np.zeros((B, NQ, D), np.float32)
    for c in range(NCORE):
        part = res.results[c]["out"]  # [128 p, NDC m, B, NQ]; dout = m*128+p
        out += part.transpose(2, 3, 1, 0).reshape(B, NQ, D).astype(np.float32)
    out += bias_total[None, None, :]
    return out


# revision 8
# speedup vs baseline: 1.2739x; 1.1967x over previous
"""Masked cross-attention (B=4, NQ=100, HW=4096, D=1024, H=16) on 8 TRN2 cores.

Sharding: kv rows (keys) are split 8 ways; each core runs LayerNorm + K/V
projection on its 512-key slice per batch, computes unnormalized partial
attention for all (b, h) against its keys, all-reduces the softmax
denominators on device, normalizes, and computes a partial out-projection.
The host sums the 8 partial outputs and adds the folded bias.

Schedule notes (v2):
 - q + wq load first; Q projection is the first PE work (warms the HAM
   clock gate while kv0/wk stream in), so the first matmul fires ~15us in
   instead of ~40us.
 - ALL kv loads and XBAR transposes are emitted before the first
   collective: any DMA emitted after collective K in program order waits
   for K to complete (one program-order CC counter), which in the old
   schedule stalled kvnT3's transpose ~13us and combine2's transpose
   ~19us.
 - A warm-up AllReduce on a constant fires as soon as the gpsimd queue
   starts, absorbing cross-core launch skew before the first real
   (data-dependent) denominator all-reduce.
 - The mask is applied as an additive -1e9 bias on the DVE directly into
   the scores PSUM before the exp, instead of a post-exp multiply in
   SBUF: fewer SBUF touches next to the streaming matmuls.
 - recip fetches / combine transposes / output DMAs are emitted before
   later all-reduces so only batch 3's combine is gated by the last
   collective.

LayerNorm gamma/beta are folded into the projection weights/biases on the
host; the V-projection bias is folded into the final output bias (exact
because softmax weights sum to one) and the K-projection bias is dropped
entirely (it shifts every key score of a query equally, which softmax
cancels).
"""
import sys

sys.path.insert(0, "/opt/trn_rl_repo")

import numpy as np
import ml_dtypes

import concourse.bacc as bacc
import concourse.bass as bass
import concourse.mybir as mybir
import concourse.tile as tile
from concourse.bass_utils import run_bass_kernel_spmd
B, NQ, HW, D, H = 4, 100, 4096, 1024, 16
HD = D // H          # 64
NCORE = 8
KC = HW // NCORE     # 512 keys per core per batch
NKT = KC // 128      # 4 key sub-tiles of 128
NDC = D // 128       # 8 chunks of the model dim
EPS = 1e-5
SCALE = 1.0 / np.sqrt(np.float32(HD))  # 1/8

F32 = mybir.dt.float32
BF16 = mybir.dt.bfloat16
AF = mybir.ActivationFunctionType
ALU = mybir.AluOpType

_compiled = {}


def _build():
    nc = bacc.Bacc("TRN2", target_bir_lowering=False, num_devices=NCORE)

    kv_d = nc.dram_tensor("kv", [B, NKT, 128, D], BF16, kind="ExternalInput")
    q_d = nc.dram_tensor("q", [B, NQ, D], BF16, kind="ExternalInput")
    # additive mask bias (0 or -1e9), duplicated over the i=2 head-pair dim
    mask_d = nc.dram_tensor("maskT", [128, B, NKT, 2, NQ], BF16,
                            kind="ExternalInput")
    wq_d = nc.dram_tensor("wqT", [128, NDC, D], BF16, kind="ExternalInput")
    wk_d = nc.dram_tensor("wkT", [128, NDC, D], BF16, kind="ExternalInput")
    wv_d = nc.dram_tensor("wvT", [128, NDC, D], BF16, kind="ExternalInput")
    wo_d = nc.dram_tensor("woT", [128, NDC, D], BF16, kind="ExternalInput")
    bq_d = nc.dram_tensor("biasq", [128, NDC], F32, kind="ExternalInput")
    out_d = nc.dram_tensor("out", [128, NDC, B, NQ], BF16, kind="ExternalOutput")

    with tile.TileContext(nc) as tc:
        with (
            tc.tile_pool(name="sb", bufs=1) as sb,
            tc.tile_pool(name="ps", bufs=1, space="PSUM") as ps,
            tc.tile_pool(name="dram", bufs=1, space="DRAM") as dram,
        ):
            # ---- constants ----
            eps_t = sb.tile([128, 1], F32, tag="eps")
            nc.vector.memset(eps_t[:], EPS)
            warm = sb.tile([1, 4], F32, tag="actwarm")
            nc.vector.memset(warm[:], 1.0)
            nc.scalar.activation(warm[:1, 0:1], warm[:1, 2:3], AF.Sqrt)
            nc.scalar.activation(warm[:1, 0:1], warm[:1, 2:3], AF.Copy)
            nc.scalar.activation(warm[:1, 0:1], warm[:1, 2:3], AF.Identity)
            nc.scalar.activation(warm[:1, 0:1], warm[:1, 2:3], AF.Exp)

            wk_sb = sb.tile([128, NDC, D], BF16, tag="wk")
            wv_sb = sb.tile([128, NDC, D], BF16, tag="wv")
            bqv_sb = sb.tile([128, NDC], F32, tag="bqv")
            bq_sb = [bqv_sb[:, j:j + 1] for j in range(NDC)]
            wq_sb = sb.tile([128, NDC, D], BF16, tag="wq", bufs=1, name="wq")
            wo_sb = sb.tile([128, NDC, D], BF16, tag="wo")

            def layernorm_to_bf16(x_bf16, xn_bf16, p):
                """(x - mean) * rsqrt(var + eps), row-wise over the free dim."""
                stats = sb.tile([128, 2, 6], F32, tag="lnstats", bufs=4)
                nc.vector.bn_stats(stats[:p, 0, :], x_bf16[:p, 0:512])
                nc.vector.bn_stats(stats[:p, 1, :], x_bf16[:p, 512:1024])
                mv = sb.tile([128, 2], F32, tag="lnmv", bufs=4)
                nc.vector.bn_aggr(mv[:p], stats[:p])
                rstd = sb.tile([128, 1], F32, tag="lnrstd", bufs=4)
                nc.scalar.activation(rstd[:p], mv[:p, 1:2], AF.Sqrt, bias=eps_t[:p])
                nc.vector.reciprocal(rstd[:p], rstd[:p])
                nc.vector.tensor_scalar(
                    xn_bf16[:p], x_bf16[:p], mv[:p, 0:1], rstd[:p],
                    ALU.subtract, ALU.mult,
                )

            sloc = [dram.tile([NQ, H], F32, tag=f"sloc{b}", name=f"sloc{b}")
                    for b in range(B)]
            sglob = [dram.tile([NQ, H], F32, tag=f"sglob{b}", name=f"sglob{b}")
                     for b in range(B)]
            # normalized, transposed context for all batches: [p, b, k, q]
            ctxT_all = sb.tile([128, B, NDC, 112], BF16, tag="ctxTall")
            NQP = 112  # q rows padded to the XBAR 16-row granule
            qnT = sb.tile([128, B, NDC, NQP], BF16, tag="qnT")
            qpT = []

            def load_kv(b):
                kvraws = []
                for r in range(NKT):
                    kvraw = sb.tile([128, D], BF16, tag="kvraw", bufs=4)
                    nc.sync.dma_start(kvraw[:], kv_d[b, r])
                    kvraws.append(kvraw)
                return kvraws

            def transpose_kv(b, kvraws):
                """LayerNorm + XBAR DMA transpose into kvnT[b].

                kvnT layout: [128 dpart, NKT, NDC, 128 keys]."""
                kvnT = sb.tile([128, NKT, NDC, 128], BF16, tag="kvnT", bufs=3,
                               name=f"kvnT_{b}")
                for r in range(NKT):
                    layernorm_to_bf16(kvraws[r], kvraws[r], 128)
                    nc.scalar.dma_start_transpose(kvnT[:, r], kvraws[r][:])
                return kvnT

            def kproj(b, kvnT):
                """K projection -> kpT[j]: [128 dout, KC keys] (no bias)."""
                kpT = []
                for j in range(NDC):
                    kpT.append(
                        sb.tile([128, KC], BF16, tag=f"kpT{j}", bufs=2,
                                name=f"kpT{j}_{b}")
                    )
                    acc = ps.tile([128, KC], F32, tag="mm", bufs=3)
                    for k in range(NDC):
                        nc.tensor.matmul(
                            acc[:],
                            lhsT=wk_sb[:, k, j * 128:(j + 1) * 128],
                            rhs=kvnT[:, :, k, :],
                            start=(k == 0), stop=(k == NDC - 1),
                        )
                    nc.scalar.activation(kpT[j][:], acc[:], AF.Copy)
                return kpT

            def vproj(b, kvnT):
                """V projection -> vp_ext[r]: [128 keys, H, HD+1], col HD=1."""
                vp_ext = []
                for r in range(NKT):
                    vpe = sb.tile([128, H, HD + 1], BF16, tag=f"vpe{r}", bufs=2,
                                  name=f"vpe{r}_{b}")
                    vp_ext.append(vpe)
                    nc.vector.memset(vpe[:, :, HD:HD + 1], 1.0)
                    for nh in range(2):
                        acc = ps.tile([128, 512], F32, tag="mm", bufs=3)
                        for k in range(NDC):
                            nc.tensor.matmul(
                                acc[:],
                                lhsT=kvnT[:, r, k, :],
                                rhs=wv_sb[:, k, nh * 512:(nh + 1) * 512],
                                start=(k == 0), stop=(k == NDC - 1),
                            )
                        if nh == 0:
                            nc.vector.tensor_copy(
                                out=vpe[:, nh * 8:(nh + 1) * 8, 0:HD],
                                in_=acc[:].rearrange("p (g d) -> p g d", g=8),
                            )
                        else:
                            nc.scalar.activation(
                                vpe[:, nh * 8:(nh + 1) * 8, 0:HD],
                                acc[:].rearrange("p (g d) -> p g d", g=8),
                                AF.Copy,
                            )
                return vp_ext

            def load_q():
                qraws = []
                for b in range(B):
                    qraw = sb.tile([NQ, D], BF16, tag="qraw", bufs=2)
                    nc.sync.dma_start(qraw[:], q_d[b])
                    qraws.append(qraw)
                return qraws

            def ln_T_q1(qraw, b):
                qn = sb.tile([NQP, D], BF16, tag="qn", bufs=2)
                layernorm_to_bf16(qraw, qn, NQ)
                nc.scalar.dma_start_transpose(qnT[:, b], qn[:])

            for j in range(NDC):
                qpT.append(
                    sb.tile([128, B, 2, NQ], BF16, tag=f"qpT{j}",
                            name=f"qpT{j}")
                )

            def qproj(b0):
                """qpT_pad[j][:, b0:b0+2]: block-diagonal by head, for one
                batch pair (lets scores(0) start after only two q
                transposes)."""
                for j in range(NDC):
                    if b0 == 0:
                        nc.gpsimd.memset(qpT[j][:], 0.0)
                    acc = ps.tile([128, 2 * NQ], F32, tag="sc", bufs=2)
                    for k in range(NDC):
                        nc.tensor.matmul(
                            acc[:],
                            lhsT=wq_sb[:, k, j * 128:(j + 1) * 128],
                            rhs=qnT[:, b0:b0 + 2, k, 0:NQ],
                            start=(k == 0), stop=(k == NDC - 1),
                        )
                    nc.scalar.activation(
                        qpT[j][0:HD, b0:b0 + 2, 0, :],
                        acc[0:HD, :].rearrange("p (b q) -> p b q", b=2),
                        AF.Identity, bias=bq_sb[j][0:HD],
                    )
                    nc.scalar.activation(
                        qpT[j][HD:128, b0:b0 + 2, 1, :],
                        acc[HD:128, :].rearrange("p (b q) -> p b q", b=2),
                        AF.Identity, bias=bq_sb[j][HD:128],
                    )

            mask_all = sb.tile([128, B, NKT, 2, NQ], BF16, tag="maskb")

            def scores_exp(b, kpT):
                """scores^T + additive mask bias + exp for all heads of b."""
                exp_all = sb.tile([128, NKT, H, NQ], BF16, tag="expall", bufs=2,
                                  name=f"exp_{b}")
                for j in range(NDC):
                    for c2 in range(2):
                        sc = ps.tile([128, 2, 2, NQ], F32, tag="sc", bufs=2)
                        for cc in range(2):
                            c = 2 * c2 + cc
                            nc.tensor.matmul(
                                sc[:, cc, :, :].rearrange("p i q -> p (i q)"),
                                lhsT=kpT[j][:, c * 128:(c + 1) * 128],
                                rhs=qpT[j][:, b, :, :].rearrange(
                                    "p i q -> p (i q)"),
                                start=True, stop=True,
                            )
                        for cc in range(2):
                            c = 2 * c2 + cc
                            nc.vector.tensor_add(
                                sc[:, cc, :, :], sc[:, cc, :, :],
                                mask_all[:, b, c, :, :],
                            )
                        nc.scalar.activation(
                            exp_all[:, 2 * c2:2 * c2 + 2, 2 * j:2 * j + 2, :],
                            sc[:], AF.Exp, scale=float(SCALE),
                        )
                return exp_all

            def ctx_block(b, exp_all, vp_ext):
                """Unnormalized ctx + denominators for batch b; DMA the local
                denominators out for the all-reduce."""
                ctx_b = sb.tile([NQ, H, HD + 1], BF16, tag="ctxsb", bufs=4,
                                name=f"ctx_{b}")
                for h in range(H):
                    ctx_ps = ps.tile([NQ, HD + 1], F32, tag="small", bufs=3)
                    for c in range(NKT):
                        nc.tensor.matmul(
                            ctx_ps[:],
                            lhsT=exp_all[:, c, h, :],
                            rhs=vp_ext[c][:, h, :],
                            start=(c == 0), stop=(c == NKT - 1),
                        )
                    if h % 2 == 0:
                        nc.vector.tensor_copy(out=ctx_b[:, h, :], in_=ctx_ps[:])
                    else:
                        nc.scalar.activation(ctx_b[:, h, :], ctx_ps[:], AF.Copy)
                den = sb.tile([NQ, H], F32, tag="den", bufs=4)
                nc.gpsimd.tensor_copy(out=den[:], in_=ctx_b[:, :, HD])
                nc.sync.dma_start(sloc[b][:], den[:])
                return ctx_b

            def allreduce_b(b):
                nc.gpsimd.collective_compute(
                    "AllReduce", ALU.add,
                    replica_groups=[list(range(NCORE))],
                    ins=[sloc[b][:].opt()], outs=[sglob[b][:].opt()],
                )
                return b

            def recip_fetch(b):
                recip = sb.tile([NQ, H], F32, tag="recip", bufs=4,
                                name=f"recip_{b}")
                nc.scalar.dma_start(recip[:], sglob[b][:])
                return recip

            def combine_block(b, ctx_b, recip):
                """Normalize by global denominators and XBAR-transpose into
                ctxT_all."""
                nc.vector.reciprocal(recip[:], recip[:])
                ctxn = sb.tile([112, H, HD], BF16, tag="ctxn", bufs=1)
                for h in range(H):
                    if h % 2 == 0:
                        nc.vector.tensor_scalar_mul(
                            ctxn[:NQ, h, :], ctx_b[:, h, 0:HD],
                            recip[:, h:h + 1]
                        )
                    else:
                        nc.scalar.activation(
                            ctxn[:NQ, h, :], ctx_b[:, h, 0:HD], AF.Copy,
                            scale=recip[:, h:h + 1],
                        )
                nc.scalar.dma_start_transpose(ctxT_all[:, b], ctxn[:])

            # reuses wq's slot (wq is dead after the Q projection)
            out_sb = sb.tile([128, NDC, B, NQ], BF16, tag="wq", bufs=1,
                             name="out_sb")

            def outproj(b0, nb):
                for m in range(NDC):
                    acc = ps.tile([128, nb, NQ], F32, tag="sc", bufs=2)
                    for k in range(NDC):
                        nc.tensor.matmul(
                            acc[:],
                            lhsT=wo_sb[:, k, m * 128:(m + 1) * 128],
                            rhs=ctxT_all[:, b0:b0 + nb, k, 0:NQ],
                            start=(k == 0), stop=(k == NDC - 1),
                        )
                    if m % 2 == 0:
                        nc.vector.tensor_copy(
                            out=out_sb[:, m, b0:b0 + nb, :], in_=acc[:]
                        )
                    else:
                        nc.scalar.activation(
                            out_sb[:, m, b0:b0 + nb, :], acc[:], AF.Copy,
                        )
                nc.sync.dma_start(
                    out_d[:, :, b0:b0 + nb, :], out_sb[:, :, b0:b0 + nb, :]
                )

            # ---- pipelined schedule (v3) ----
            # Sync-ring FIFO: wq (per-chunk so qproj j0 starts early), q,
            # bq, then kv0+wk, then the rest; denominator DMAs come after
            # every load and before the output DMAs.  ACT-ring FIFO: q0/q1
            # transposes first (qproj pair 0 gates the first matmuls), kv
            # transposes next, recips/ctxT last.  All kv transposes are
            # emitted before the first all-reduce.
            for j in range(NDC):
                nc.sync.dma_start(
                    wq_sb[:, :, j * 128:(j + 1) * 128],
                    wq_d[:, :, j * 128:(j + 1) * 128],
                )
            qraws = load_q()
            nc.sync.dma_start(bqv_sb[:], bq_d[:])
            kvraws0 = load_kv(0)
            nc.sync.dma_start(wk_sb[:], wk_d[:])

            ln_T_q1(qraws[0], 0)
            ln_T_q1(qraws[1], 1)
            qproj(0)

            kvnT0 = transpose_kv(0, kvraws0)
            ln_T_q1(qraws[2], 2)
            ln_T_q1(qraws[3], 3)
            kvraws1 = load_kv(1)
            nc.sync.dma_start(wv_sb[:], wv_d[:])
            nc.sync.dma_start(mask_all[:], mask_d[:])

            kpT0 = kproj(0, kvnT0)
            qproj(2)
            kvnT1 = transpose_kv(1, kvraws1)
            kvraws2 = load_kv(2)
            nc.sync.dma_start(wo_sb[:], wo_d[:])
            vp0 = vproj(0, kvnT0)
            kvnT2 = transpose_kv(2, kvraws2)
            kvraws3 = load_kv(3)
            kvnT3 = transpose_kv(3, kvraws3)

            exp0 = scores_exp(0, kpT0)
            kpT1 = kproj(1, kvnT1)
            ctx0 = ctx_block(0, exp0, vp0)
            allreduce_b(0)
            r0 = recip_fetch(0)
            combine_block(0, ctx0, r0)
            vp1 = vproj(1, kvnT1)

            exp1 = scores_exp(1, kpT1)
            kpT2 = kproj(2, kvnT2)
            ctx1 = ctx_block(1, exp1, vp1)
            allreduce_b(1)
            r1 = recip_fetch(1)
            combine_block(1, ctx1, r1)
            vp2 = vproj(2, kvnT2)

            exp2 = scores_exp(2, kpT2)
            kpT3 = kproj(3, kvnT3)
            ctx2 = ctx_block(2, exp2, vp2)
            allreduce_b(2)
            r2 = recip_fetch(2)
            combine_block(2, ctx2, r2)
            vp3 = vproj(3, kvnT3)

            exp3 = scores_exp(3, kpT3)
            outproj(0, 2)
            ctx3 = ctx_block(3, exp3, vp3)
            allreduce_b(3)
            r3 = recip_fetch(3)
            combine_block(3, ctx3, r3)
            outproj(2, 1)
            outproj(3, 1)

    nc.compile()
    return nc


def _prep_in_maps(q, kv, mask, in_proj_w, in_proj_b, out_w, out_b,
                  g_q, b_q, g_kv, b_kv):
    """Host-side prep: fold LN affine + V-bias, shard kv/mask per core.

    Returns (in_maps, bias_total)."""
    q = np.asarray(q, np.float32)
    kv = np.asarray(kv, np.float32)
    mask = np.asarray(mask)
    in_proj_w = np.asarray(in_proj_w, np.float32)
    in_proj_b = np.asarray(in_proj_b, np.float32)
    out_w = np.asarray(out_w, np.float32)
    out_b = np.asarray(out_b, np.float32)
    g_q = np.asarray(g_q, np.float32)
    b_q = np.asarray(b_q, np.float32)
    g_kv = np.asarray(g_kv, np.float32)
    b_kv = np.asarray(b_kv, np.float32)

    Wq, Wk, Wv = in_proj_w[:D], in_proj_w[D:2 * D], in_proj_w[2 * D:]
    bq, bk, bv = in_proj_b[:D], in_proj_b[D:2 * D], in_proj_b[2 * D:]

    # Fold LayerNorm affine into projections: LN(x)*g+b @ W^T + c
    #   = LN(x) @ (W*g)^T + (W@b + c)
    WqT = (Wq * g_q[None, :]).T.astype(ml_dtypes.bfloat16)
    WkT = (Wk * g_kv[None, :]).T.astype(ml_dtypes.bfloat16)
    WvT = (Wv * g_kv[None, :]).T.astype(ml_dtypes.bfloat16)
    bq_eff = (bq + Wq @ b_q).astype(np.float32)
    bv_eff = (bv + Wv @ b_kv).astype(np.float32)
    # K bias cancels in softmax; V bias folds into the output bias.
    WoT = out_w.T.astype(ml_dtypes.bfloat16)
    bias_total = (out_b + out_w @ bv_eff).astype(np.float32)

    # per-query key mask; all-zero mask rows attend everywhere
    kv16 = kv.astype(ml_dtypes.bfloat16)
    allowed = (mask != 0)
    has_any = allowed.any(axis=-1, keepdims=True)
    eff = np.where(has_any, allowed, True)  # [B, NQ, HW] bool

    common = {
        "q": np.ascontiguousarray(q.astype(ml_dtypes.bfloat16)),
        "wqT": np.ascontiguousarray(WqT.reshape(NDC, 128, D).transpose(1, 0, 2)),
        "wkT": np.ascontiguousarray(WkT.reshape(NDC, 128, D).transpose(1, 0, 2)),
        "wvT": np.ascontiguousarray(WvT.reshape(NDC, 128, D).transpose(1, 0, 2)),
        "woT": np.ascontiguousarray(WoT.reshape(NDC, 128, D).transpose(1, 0, 2)),
        "biasq": np.ascontiguousarray(bq_eff.reshape(NDC, 128).T),
    }
    in_maps = []
    for c in range(NCORE):
        sl = slice(c * KC, (c + 1) * KC)
        kv_c = kv16[:, sl, :].reshape(B, NKT, 128, D)
        # additive mask bias slice -> [128, B, NKT, 2, NQ] bf16
        m_c = eff[:, :, sl].transpose(0, 2, 1).reshape(B, NKT, 128, NQ)
        mb = np.where(m_c, np.float32(0.0), np.float32(-1e9))
        mb = mb.transpose(2, 0, 1, 3)  # [128, B, NKT, NQ]
        mb = np.broadcast_to(mb[:, :, :, None, :], (128, B, NKT, 2, NQ))
        in_maps.append({
            **common,
            "kv": np.ascontiguousarray(kv_c),
            "maskT": np.ascontiguousarray(mb.astype(ml_dtypes.bfloat16)),
        })
    return in_maps, bias_total


def kernel(q, kv, mask, in_proj_w, in_proj_b, out_w, out_b, g_q, b_q, g_kv, b_kv):
    in_maps, bias_total = _prep_in_maps(
        q, kv, mask, in_proj_w, in_proj_b, out_w, out_b, g_q, b_q, g_kv, b_kv
    )
    if "nc" not in _compiled:
        _compiled["nc"] = _build()
    nc = _compiled["nc"]

    res = run_bass_kernel_spmd(nc, in_maps, core_ids=list(range(NCORE)))

    out = np.zeros((B, NQ, D), np.float32)
    for c in range(NCORE):
        part = res.results[c]["out"]  # [128 p, NDC m, B, NQ]; dout = m*128+p
        out += part.transpose(2, 3, 1, 0).reshape(B, NQ, D).astype(np.float32)
    out += bias_total[None, None, :]
    return out


# revision 11
# speedup vs baseline: 1.4208x; 1.1153x over previous
"""Masked cross-attention (B=4, NQ=100, HW=4096, D=1024, H=16) on 8 TRN2 cores.

Sharding: kv rows (keys) are split 8 ways; each core runs LayerNorm + K/V
projection on its 512-key slice per batch, computes unnormalized partial
attention for all (b, h) against its keys, all-reduces the softmax
denominators on device, normalizes, and computes a partial out-projection.
The host sums the 8 partial outputs and adds the folded bias.

Schedule notes (v2):
 - q + wq load first; Q projection is the first PE work (warms the HAM
   clock gate while kv0/wk stream in), so the first matmul fires ~15us in
   instead of ~40us.
 - ALL kv loads and XBAR transposes are emitted before the first
   collective: any DMA emitted after collective K in program order waits
   for K to complete (one program-order CC counter), which in the old
   schedule stalled kvnT3's transpose ~13us and combine2's transpose
   ~19us.
 - A warm-up AllReduce on a constant fires as soon as the gpsimd queue
   starts, absorbing cross-core launch skew before the first real
   (data-dependent) denominator all-reduce.
 - The mask is applied as an additive -1e9 bias on the DVE directly into
   the scores PSUM before the exp, instead of a post-exp multiply in
   SBUF: fewer SBUF touches next to the streaming matmuls.
 - recip fetches / combine transposes / output DMAs are emitted before
   later all-reduces so only batch 3's combine is gated by the last
   collective.

LayerNorm gamma/beta are folded into the projection weights/biases on the
host; the V-projection bias is folded into the final output bias (exact
because softmax weights sum to one) and the K-projection bias is dropped
entirely (it shifts every key score of a query equally, which softmax
cancels).
"""
import sys

sys.path.insert(0, "/opt/trn_rl_repo")

import numpy as np
import ml_dtypes

import concourse.bacc as bacc
import concourse.bass as bass
import concourse.mybir as mybir
import concourse.tile as tile
from concourse.bass_utils import run_bass_kernel_spmd
B, NQ, HW, D, H = 4, 100, 4096, 1024, 16
HD = D // H          # 64
NCORE = 8
KC = HW // NCORE     # 512 keys per core per batch
NKT = KC // 128      # 4 key sub-tiles of 128
NDC = D // 128       # 8 chunks of the model dim
EPS = 1e-5
SCALE = 1.0 / np.sqrt(np.float32(HD))  # 1/8

F32 = mybir.dt.float32
BF16 = mybir.dt.bfloat16
AF = mybir.ActivationFunctionType
ALU = mybir.AluOpType

_compiled = {}


def _build():
    nc = bacc.Bacc("TRN2", target_bir_lowering=False, num_devices=NCORE)

    kv_d = nc.dram_tensor("kv", [B, NKT, 128, D], BF16, kind="ExternalInput")
    q_d = nc.dram_tensor("q", [B, NQ, D], BF16, kind="ExternalInput")
    # additive mask bias (0 or -1e9), duplicated over the i=2 head-pair dim
    mask_d = nc.dram_tensor("maskT", [128, B, NKT, 2, NQ], BF16,
                            kind="ExternalInput")
    wq_d = nc.dram_tensor("wqT", [128, NDC, D], BF16, kind="ExternalInput")
    wk_d = nc.dram_tensor("wkT", [128, NDC, D], BF16, kind="ExternalInput")
    wv_d = nc.dram_tensor("wvT", [128, NDC, D], BF16, kind="ExternalInput")
    wo_d = nc.dram_tensor("woT", [128, NDC, D], BF16, kind="ExternalInput")
    bq_d = nc.dram_tensor("biasq", [128, NDC], F32, kind="ExternalInput")
    out_d = nc.dram_tensor("out", [128, NDC, B, NQ], BF16, kind="ExternalOutput")

    with tile.TileContext(nc) as tc:
        with (
            tc.tile_pool(name="sb", bufs=1) as sb,
            tc.tile_pool(name="ps", bufs=1, space="PSUM") as ps,
            tc.tile_pool(name="dram", bufs=1, space="DRAM") as dram,
        ):
            # ---- constants ----
            eps_t = sb.tile([128, 1], F32, tag="eps")
            nc.vector.memset(eps_t[:], EPS)
            warm = sb.tile([1, 4], F32, tag="actwarm")
            nc.vector.memset(warm[:], 1.0)
            nc.scalar.activation(warm[:1, 0:1], warm[:1, 2:3], AF.Sqrt)
            nc.scalar.activation(warm[:1, 0:1], warm[:1, 2:3], AF.Copy)
            nc.scalar.activation(warm[:1, 0:1], warm[:1, 2:3], AF.Identity)
            nc.scalar.activation(warm[:1, 0:1], warm[:1, 2:3], AF.Exp)

            wk_sb = sb.tile([128, NDC, D], BF16, tag="wk")
            wv_sb = sb.tile([128, NDC, D], BF16, tag="wv")
            bqv_sb = sb.tile([128, NDC], F32, tag="bqv")
            bq_sb = [bqv_sb[:, j:j + 1] for j in range(NDC)]
            wq_sb = sb.tile([128, NDC, D], BF16, tag="wq", bufs=1, name="wq")
            wo_sb = sb.tile([128, NDC, D], BF16, tag="wo")

            def layernorm_to_bf16(x_bf16, xn_bf16, p):
                """(x - mean) * rsqrt(var + eps), row-wise over the free dim."""
                stats = sb.tile([128, 2, 6], F32, tag="lnstats", bufs=2)
                nc.vector.bn_stats(stats[:p, 0, :], x_bf16[:p, 0:512])
                nc.vector.bn_stats(stats[:p, 1, :], x_bf16[:p, 512:1024])
                mv = sb.tile([128, 2], F32, tag="lnmv", bufs=2)
                nc.vector.bn_aggr(mv[:p], stats[:p])
                rstd = sb.tile([128, 1], F32, tag="lnrstd", bufs=2)
                nc.scalar.activation(rstd[:p], mv[:p, 1:2], AF.Sqrt, bias=eps_t[:p])
                nc.vector.reciprocal(rstd[:p], rstd[:p])
                nc.vector.tensor_scalar(
                    xn_bf16[:p], x_bf16[:p], mv[:p, 0:1], rstd[:p],
                    ALU.subtract, ALU.mult,
                )

            sloc = [dram.tile([NQ, H], F32, tag=f"sloc{b}", name=f"sloc{b}")
                    for b in range(B)]
            sglob = [dram.tile([NQ, H], F32, tag=f"sglob{b}", name=f"sglob{b}")
                     for b in range(B)]
            # normalized, transposed context for all batches: [p, b, k, q]
            ctxT_all = sb.tile([128, B, NDC, 112], BF16, tag="ctxTall")
            NQP = 112  # q rows padded to the XBAR 16-row granule
            qnT = sb.tile([128, B, NDC, NQP], BF16, tag="qnT")
            qpT = []

            def load_kv(b):
                kvraws = []
                for r in range(NKT):
                    kvraw = sb.tile([128, D], BF16, tag="kvraw", bufs=4)
                    nc.sync.dma_start(kvraw[:], kv_d[b, r])
                    kvraws.append(kvraw)
                return kvraws

            def transpose_kv(b, kvraws):
                """LayerNorm + XBAR DMA transpose into kvnT[b].

                kvnT layout: [128 dpart, NKT, NDC, 128 keys]."""
                kvnT = sb.tile([128, NKT, NDC, 128], BF16, tag="kvnT", bufs=3,
                               name=f"kvnT_{b}")
                for r in range(NKT):
                    layernorm_to_bf16(kvraws[r], kvraws[r], 128)
                    nc.sync.dma_start_transpose(kvnT[:, r], kvraws[r][:])
                return kvnT

            def kproj(b, kvnT):
                """K projection -> kpT[j]: [128 dout, KC keys] (no bias)."""
                kpT = []
                for j in range(NDC):
                    kpT.append(
                        sb.tile([128, KC], BF16, tag=f"kpT{j}", bufs=2,
                                name=f"kpT{j}_{b}")
                    )
                    acc = ps.tile([128, KC], F32, tag="mm", bufs=3)
                    for k in range(NDC):
                        nc.tensor.matmul(
                            acc[:],
                            lhsT=wk_sb[:, k, j * 128:(j + 1) * 128],
                            rhs=kvnT[:, :, k, :],
                            start=(k == 0), stop=(k == NDC - 1),
                        )
                    nc.scalar.activation(kpT[j][:], acc[:], AF.Copy)
                return kpT

            def vproj(b, kvnT):
                """V projection -> vp_ext[r]: [128 keys, H, HD+1], col HD=1."""
                vp_ext = []
                for r in range(NKT):
                    vpe = sb.tile([128, H, HD + 1], BF16, tag=f"vpe{r}", bufs=2,
                                  name=f"vpe{r}_{b}")
                    vp_ext.append(vpe)
                    nc.vector.memset(vpe[:, :, HD:HD + 1], 1.0)
                    for nh in range(2):
                        acc = ps.tile([128, 512], F32, tag="mm", bufs=3)
                        for k in range(NDC):
                            nc.tensor.matmul(
                                acc[:],
                                lhsT=kvnT[:, r, k, :],
                                rhs=wv_sb[:, k, nh * 512:(nh + 1) * 512],
                                start=(k == 0), stop=(k == NDC - 1),
                            )
                        if nh == 0:
                            nc.vector.tensor_copy(
                                out=vpe[:, nh * 8:(nh + 1) * 8, 0:HD],
                                in_=acc[:].rearrange("p (g d) -> p g d", g=8),
                            )
                        else:
                            nc.scalar.activation(
                                vpe[:, nh * 8:(nh + 1) * 8, 0:HD],
                                acc[:].rearrange("p (g d) -> p g d", g=8),
                                AF.Copy,
                            )
                return vp_ext

            def load_q():
                qraws = []
                for b in range(B):
                    qraw = sb.tile([NQ, D], BF16, tag="qraw", bufs=2)
                    nc.sync.dma_start(qraw[:], q_d[b])
                    qraws.append(qraw)
                return qraws

            def ln_T_q1(qraw, b):
                qn = sb.tile([NQP, D], BF16, tag="qn", bufs=2)
                layernorm_to_bf16(qraw, qn, NQ)
                nc.sync.dma_start_transpose(qnT[:, b], qn[:])

            for j in range(NDC):
                qpT.append(
                    sb.tile([128, B, 2, NQ], BF16, tag=f"qpT{j}",
                            name=f"qpT{j}")
                )

            def qproj(b0):
                """qpT_pad[j][:, b0:b0+2]: block-diagonal by head, for one
                batch pair (lets scores(0) start after only two q
                transposes)."""
                for j in range(NDC):
                    if b0 == 0:
                        nc.gpsimd.memset(qpT[j][:], 0.0)
                    acc = ps.tile([128, 2 * NQ], F32, tag="sc", bufs=2)
                    for k in range(NDC):
                        nc.tensor.matmul(
                            acc[:],
                            lhsT=wq_sb[:, k, j * 128:(j + 1) * 128],
                            rhs=qnT[:, b0:b0 + 2, k, 0:NQ],
                            start=(k == 0), stop=(k == NDC - 1),
                        )
                    nc.scalar.activation(
                        qpT[j][0:HD, b0:b0 + 2, 0, :],
                        acc[0:HD, :].rearrange("p (b q) -> p b q", b=2),
                        AF.Identity, bias=bq_sb[j][0:HD],
                    )
                    nc.scalar.activation(
                        qpT[j][HD:128, b0:b0 + 2, 1, :],
                        acc[HD:128, :].rearrange("p (b q) -> p b q", b=2),
                        AF.Identity, bias=bq_sb[j][HD:128],
                    )

            mask_all = sb.tile([128, B, NKT, 2, NQ], BF16, tag="maskb")

            def scores_exp(b, kpT):
                """scores^T + additive mask bias + exp for all heads of b."""
                exp_all = sb.tile([128, NKT, H, NQ], BF16, tag="expall", bufs=2,
                                  name=f"exp_{b}")
                for j in range(NDC):
                    for c2 in range(2):
                        sc = ps.tile([128, 2, 2, NQ], F32, tag="sc", bufs=2)
                        for cc in range(2):
                            c = 2 * c2 + cc
                            nc.tensor.matmul(
                                sc[:, cc, :, :].rearrange("p i q -> p (i q)"),
                                lhsT=kpT[j][:, c * 128:(c + 1) * 128],
                                rhs=qpT[j][:, b, :, :].rearrange(
                                    "p i q -> p (i q)"),
                                start=True, stop=True,
                            )
                        for cc in range(2):
                            c = 2 * c2 + cc
                            nc.vector.tensor_add(
                                sc[:, cc, :, :], sc[:, cc, :, :],
                                mask_all[:, b, c, :, :],
                            )
                        nc.scalar.activation(
                            exp_all[:, 2 * c2:2 * c2 + 2, 2 * j:2 * j + 2, :],
                            sc[:], AF.Exp, scale=float(SCALE),
                        )
                return exp_all

            def ctx_block(b, exp_all, vp_ext):
                """Unnormalized ctx + denominators for batch b; DMA the local
                denominators out for the all-reduce."""
                ctx_b = sb.tile([NQ, H, HD + 1], BF16, tag="ctxsb", bufs=4,
                                name=f"ctx_{b}")
                for h in range(H):
                    ctx_ps = ps.tile([NQ, HD + 1], F32, tag="small", bufs=3)
                    for c in range(NKT):
                        nc.tensor.matmul(
                            ctx_ps[:],
                            lhsT=exp_all[:, c, h, :],
                            rhs=vp_ext[c][:, h, :],
                            start=(c == 0), stop=(c == NKT - 1),
                        )
                    if h % 2 == 0:
                        nc.vector.tensor_copy(out=ctx_b[:, h, :], in_=ctx_ps[:])
                    else:
                        nc.scalar.activation(ctx_b[:, h, :], ctx_ps[:], AF.Copy)
                den = sb.tile([NQ, H], F32, tag="den", bufs=2)
                nc.gpsimd.tensor_copy(out=den[:], in_=ctx_b[:, :, HD])
                nc.sync.dma_start(sloc[b][:], den[:])
                return ctx_b

            def allreduce_b(b):
                nc.gpsimd.collective_compute(
                    "AllReduce", ALU.add,
                    replica_groups=[list(range(NCORE))],
                    ins=[sloc[b][:].opt()], outs=[sglob[b][:].opt()],
                )
                return b

            def recip_fetch(b):
                recip = sb.tile([NQ, H], F32, tag="recip", bufs=4,
                                name=f"recip_{b}")
                nc.scalar.dma_start(recip[:], sglob[b][:])
                return recip

            def combine_block(b, ctx_b, recip):
                """Normalize by global denominators and XBAR-transpose into
                ctxT_all."""
                nc.vector.reciprocal(recip[:], recip[:])
                ctxn = sb.tile([112, H, HD], BF16, tag="ctxn", bufs=1)
                for h in range(H):
                    if h % 2 == 0:
                        nc.vector.tensor_scalar_mul(
                            ctxn[:NQ, h, :], ctx_b[:, h, 0:HD],
                            recip[:, h:h + 1]
                        )
                    else:
                        nc.scalar.activation(
                            ctxn[:NQ, h, :], ctx_b[:, h, 0:HD], AF.Copy,
                            scale=recip[:, h:h + 1],
                        )
                nc.scalar.dma_start_transpose(ctxT_all[:, b], ctxn[:])

            # reuses wq's slot (wq is dead after the Q projection)
            out_sb = sb.tile([128, NDC, B, NQ], BF16, tag="wq", bufs=1,
                             name="out_sb")

            def outproj(b0, nb):
                for m in range(NDC):
                    acc = ps.tile([128, nb, NQ], F32, tag="sc", bufs=2)
                    for k in range(NDC):
                        nc.tensor.matmul(
                            acc[:],
                            lhsT=wo_sb[:, k, m * 128:(m + 1) * 128],
                            rhs=ctxT_all[:, b0:b0 + nb, k, 0:NQ],
                            start=(k == 0), stop=(k == NDC - 1),
                        )
                    if m % 2 == 0:
                        nc.vector.tensor_copy(
                            out=out_sb[:, m, b0:b0 + nb, :], in_=acc[:]
                        )
                    else:
                        nc.scalar.activation(
                            out_sb[:, m, b0:b0 + nb, :], acc[:], AF.Copy,
                        )
                nc.sync.dma_start(
                    out_d[:, :, b0:b0 + nb, :], out_sb[:, :, b0:b0 + nb, :]
                )

            # ---- pipelined schedule (v4) ----
            # Laws this schedule is built around (verified in traces):
            #  1. An XBAR transpose waits for every DMA emitted before it to
            #     COMPLETE (deadlock guard), and while waiting it blocks its
            #     issuing ring.  q/kv transposes therefore ride the sync
            #     ring, emitted BEFORE the loads they must not wait on; the
            #     blocking they cause there is already implied by kvraw
            #     buffer reuse.
            #  2. The ACT engine queue is strict FIFO: recip fetches are
            #     emitted after the following vproj's PSUM copies so a
            #     pending all-reduce never stalls exp/copy work.
            #  3. A collective waits for every previously emitted XBAR to
            #     complete, and DMAs emitted after a collective wait for it:
            #     all kv transposes are emitted before the first all-reduce;
            #     per-batch combine/outproj emissions trail their own
            #     all-reduce only.
            qraws = load_q()
            nc.sync.dma_start(bqv_sb[:], bq_d[:])
            ln_T_q1(qraws[0], 0)
            ln_T_q1(qraws[1], 1)
            ln_T_q1(qraws[2], 2)
            ln_T_q1(qraws[3], 3)

            kvraws0 = load_kv(0)
            nc.sync.dma_start(wk_sb[:], wk_d[:])
            kvnT0 = transpose_kv(0, kvraws0)
            for j in range(NDC):
                nc.sync.dma_start(
                    wq_sb[:, :, j * 128:(j + 1) * 128],
                    wq_d[:, :, j * 128:(j + 1) * 128],
                )
            qproj(0)

            kvraws1 = load_kv(1)
            nc.sync.dma_start(wv_sb[:], wv_d[:])
            nc.sync.dma_start(mask_all[:], mask_d[:])
            kpT0 = kproj(0, kvnT0)
            qproj(2)
            kvnT1 = transpose_kv(1, kvraws1)

            kvraws2 = load_kv(2)
            vp0 = vproj(0, kvnT0)
            kvnT2 = transpose_kv(2, kvraws2)
            kvraws3 = load_kv(3)
            nc.sync.dma_start(wo_sb[:], wo_d[:])
            kvnT3 = transpose_kv(3, kvraws3)

            exp0 = scores_exp(0, kpT0)
            kpT1 = kproj(1, kvnT1)
            ctx0 = ctx_block(0, exp0, vp0)
            allreduce_b(0)
            vp1 = vproj(1, kvnT1)
            r0 = recip_fetch(0)
            combine_block(0, ctx0, r0)

            exp1 = scores_exp(1, kpT1)
            kpT2 = kproj(2, kvnT2)
            ctx1 = ctx_block(1, exp1, vp1)
            allreduce_b(1)
            vp2 = vproj(2, kvnT2)
            r1 = recip_fetch(1)
            combine_block(1, ctx1, r1)

            exp2 = scores_exp(2, kpT2)
            kpT3 = kproj(3, kvnT3)
            ctx2 = ctx_block(2, exp2, vp2)
            allreduce_b(2)
            vp3 = vproj(3, kvnT3)
            r2 = recip_fetch(2)
            combine_block(2, ctx2, r2)

            exp3 = scores_exp(3, kpT3)
            outproj(0, 2)
            ctx3 = ctx_block(3, exp3, vp3)
            allreduce_b(3)
            r3 = recip_fetch(3)
            combine_block(3, ctx3, r3)
            outproj(2, 1)
            outproj(3, 1)

    nc.compile()
    return nc


def _prep_in_maps(q, kv, mask, in_proj_w, in_proj_b, out_w, out_b,
                  g_q, b_q, g_kv, b_kv):
    """Host-side prep: fold LN affine + V-bias, shard kv/mask per core.

    Returns (in_maps, bias_total)."""
    q = np.asarray(q, np.float32)
    kv = np.asarray(kv, np.float32)
    mask = np.asarray(mask)
    in_proj_w = np.asarray(in_proj_w, np.float32)
    in_proj_b = np.asarray(in_proj_b, np.float32)
    out_w = np.asarray(out_w, np.float32)
    out_b = np.asarray(out_b, np.float32)
    g_q = np.asarray(g_q, np.float32)
    b_q = np.asarray(b_q, np.float32)
    g_kv = np.asarray(g_kv, np.float32)
    b_kv = np.asarray(b_kv, np.float32)

    Wq, Wk, Wv = in_proj_w[:D], in_proj_w[D:2 * D], in_proj_w[2 * D:]
    bq, bk, bv = in_proj_b[:D], in_proj_b[D:2 * D], in_proj_b[2 * D:]

    # Fold LayerNorm affine into projections: LN(x)*g+b @ W^T + c
    #   = LN(x) @ (W*g)^T + (W@b + c)
    WqT = (Wq * g_q[None, :]).T.astype(ml_dtypes.bfloat16)
    WkT = (Wk * g_kv[None, :]).T.astype(ml_dtypes.bfloat16)
    WvT = (Wv * g_kv[None, :]).T.astype(ml_dtypes.bfloat16)
    bq_eff = (bq + Wq @ b_q).astype(np.float32)
    bv_eff = (bv + Wv @ b_kv).astype(np.float32)
    # K bias cancels in softmax; V bias folds into the output bias.
    WoT = out_w.T.astype(ml_dtypes.bfloat16)
    bias_total = (out_b + out_w @ bv_eff).astype(np.float32)

    # per-query key mask; all-zero mask rows attend everywhere
    kv16 = kv.astype(ml_dtypes.bfloat16)
    allowed = (mask != 0)
    has_any = allowed.any(axis=-1, keepdims=True)
    eff = np.where(has_any, allowed, True)  # [B, NQ, HW] bool

    common = {
        "q": np.ascontiguousarray(q.astype(ml_dtypes.bfloat16)),
        "wqT": np.ascontiguousarray(WqT.reshape(NDC, 128, D).transpose(1, 0, 2)),
        "wkT": np.ascontiguousarray(WkT.reshape(NDC, 128, D).transpose(1, 0, 2)),
        "wvT": np.ascontiguousarray(WvT.reshape(NDC, 128, D).transpose(1, 0, 2)),
        "woT": np.ascontiguousarray(WoT.reshape(NDC, 128, D).transpose(1, 0, 2)),
        "biasq": np.ascontiguousarray(bq_eff.reshape(NDC, 128).T),
    }
    in_maps = []
    for c in range(NCORE):
        sl = slice(c * KC, (c + 1) * KC)
        kv_c = kv16[:, sl, :].reshape(B, NKT, 128, D)
        # additive mask bias slice -> [128, B, NKT, 2, NQ] bf16
        m_c = eff[:, :, sl].transpose(0, 2, 1).reshape(B, NKT, 128, NQ)
        mb = np.where(m_c, np.float32(0.0), np.float32(-1e9))
        mb = mb.transpose(2, 0, 1, 3)  # [128, B, NKT, NQ]
        mb = np.broadcast_to(mb[:, :, :, None, :], (128, B, NKT, 2, NQ))
        in_maps.append({
            **common,
            "kv": np.ascontiguousarray(kv_c),
            "maskT": np.ascontiguousarray(mb.astype(ml_dtypes.bfloat16)),
        })
    return in_maps, bias_total


def kernel(q, kv, mask, in_proj_w, in_proj_b, out_w, out_b, g_q, b_q, g_kv, b_kv):
    in_maps, bias_total = _prep_in_maps(
        q, kv, mask, in_proj_w, in_proj_b, out_w, out_b, g_q, b_q, g_kv, b_kv
    )
    if "nc" not in _compiled:
        _compiled["nc"] = _build()
    nc = _compiled["nc"]

    res = run_bass_kernel_spmd(nc, in_maps, core_ids=list(range(NCORE)))

    out = np.zeros((B, NQ, D), np.float32)
    for c in range(NCORE):
        part = res.results[c]["out"]  # [128 p, NDC m, B, NQ]; dout = m*128+p
        out += part.transpose(2, 3, 1, 0).reshape(B, NQ, D).astype(np.float32)
    out += bias_total[None, None, :]
    return out


# revision 13
# speedup vs baseline: 1.4351x; 1.0100x over previous
"""Masked cross-attention (B=4, NQ=100, HW=4096, D=1024, H=16) on 8 TRN2 cores.

Sharding: kv rows (keys) are split 8 ways; each core runs LayerNorm + K/V
projection on its 512-key slice per batch, computes unnormalized partial
attention for all (b, h) against its keys, all-reduces the softmax
denominators on device, normalizes, and computes a partial out-projection.
The host sums the 8 partial outputs and adds the folded bias.

Schedule notes (v2):
 - q + wq load first; Q projection is the first PE work (warms the HAM
   clock gate while kv0/wk stream in), so the first matmul fires ~15us in
   instead of ~40us.
 - ALL kv loads and XBAR transposes are emitted before the first
   collective: any DMA emitted after collective K in program order waits
   for K to complete (one program-order CC counter), which in the old
   schedule stalled kvnT3's transpose ~13us and combine2's transpose
   ~19us.
 - A warm-up AllReduce on a constant fires as soon as the gpsimd queue
   starts, absorbing cross-core launch skew before the first real
   (data-dependent) denominator all-reduce.
 - The mask is applied as an additive -1e9 bias on the DVE directly into
   the scores PSUM before the exp, instead of a post-exp multiply in
   SBUF: fewer SBUF touches next to the streaming matmuls.
 - recip fetches / combine transposes / output DMAs are emitted before
   later all-reduces so only batch 3's combine is gated by the last
   collective.

LayerNorm gamma/beta are folded into the projection weights/biases on the
host; the V-projection bias is folded into the final output bias (exact
because softmax weights sum to one) and the K-projection bias is dropped
entirely (it shifts every key score of a query equally, which softmax
cancels).
"""
import sys

sys.path.insert(0, "/opt/trn_rl_repo")

import numpy as np
import ml_dtypes

import concourse.bacc as bacc
import concourse.bass as bass
import concourse.mybir as mybir
import concourse.tile as tile
from concourse.bass_utils import run_bass_kernel_spmd
B, NQ, HW, D, H = 4, 100, 4096, 1024, 16
HD = D // H          # 64
NCORE = 8
KC = HW // NCORE     # 512 keys per core per batch
NKT = KC // 128      # 4 key sub-tiles of 128
NDC = D // 128       # 8 chunks of the model dim
EPS = 1e-5
SCALE = 1.0 / np.sqrt(np.float32(HD))  # 1/8

F32 = mybir.dt.float32
BF16 = mybir.dt.bfloat16
AF = mybir.ActivationFunctionType
ALU = mybir.AluOpType

_compiled = {}


def _build():
    nc = bacc.Bacc("TRN2", target_bir_lowering=False, num_devices=NCORE)

    kv_d = nc.dram_tensor("kv", [B, NKT, 128, D], BF16, kind="ExternalInput")
    q_d = nc.dram_tensor("q", [B, NQ, D], BF16, kind="ExternalInput")
    # additive mask bias (0 or -1e9), duplicated over the i=2 head-pair dim
    mask_d = nc.dram_tensor("maskT", [128, B, NKT, 2, NQ], BF16,
                            kind="ExternalInput")
    wq_d = nc.dram_tensor("wqT", [128, NDC, D], BF16, kind="ExternalInput")
    wk_d = nc.dram_tensor("wkT", [128, NDC, D], BF16, kind="ExternalInput")
    wv_d = nc.dram_tensor("wvT", [128, NDC, D], BF16, kind="ExternalInput")
    wo_d = nc.dram_tensor("woT", [128, NDC, D], BF16, kind="ExternalInput")
    bq_d = nc.dram_tensor("biasq", [128, NDC], F32, kind="ExternalInput")
    out_d = nc.dram_tensor("out", [128, NDC, B, NQ], BF16, kind="ExternalOutput")

    with tile.TileContext(nc) as tc:
        with (
            tc.tile_pool(name="sb", bufs=1) as sb,
            tc.tile_pool(name="ps", bufs=1, space="PSUM") as ps,
            tc.tile_pool(name="dram", bufs=1, space="DRAM") as dram,
        ):
            # ---- constants ----
            eps_t = sb.tile([128, 1], F32, tag="eps")
            nc.vector.memset(eps_t[:], EPS)
            warm = sb.tile([1, 4], F32, tag="actwarm")
            nc.vector.memset(warm[:], 1.0)
            nc.scalar.activation(warm[:1, 0:1], warm[:1, 2:3], AF.Sqrt)
            nc.scalar.activation(warm[:1, 0:1], warm[:1, 2:3], AF.Copy)
            nc.scalar.activation(warm[:1, 0:1], warm[:1, 2:3], AF.Identity)
            nc.scalar.activation(warm[:1, 0:1], warm[:1, 2:3], AF.Exp)

            wk_sb = sb.tile([128, NDC, D], BF16, tag="wk")
            wv_sb = sb.tile([128, NDC, D], BF16, tag="wv")
            bqv_sb = sb.tile([128, NDC], F32, tag="bqv")
            bq_sb = [bqv_sb[:, j:j + 1] for j in range(NDC)]
            wq_sb = sb.tile([128, NDC, D], BF16, tag="wq", bufs=1, name="wq")
            wo_sb = sb.tile([128, NDC, D], BF16, tag="wo")

            def layernorm_to_bf16(x_bf16, xn_bf16, p):
                """(x - mean) * rsqrt(var + eps), row-wise over the free dim."""
                stats = sb.tile([128, 2, 6], F32, tag="lnstats", bufs=2)
                nc.vector.bn_stats(stats[:p, 0, :], x_bf16[:p, 0:512])
                nc.vector.bn_stats(stats[:p, 1, :], x_bf16[:p, 512:1024])
                mv = sb.tile([128, 2], F32, tag="lnmv", bufs=2)
                nc.vector.bn_aggr(mv[:p], stats[:p])
                rstd = sb.tile([128, 1], F32, tag="lnrstd", bufs=2)
                nc.scalar.activation(rstd[:p], mv[:p, 1:2], AF.Sqrt, bias=eps_t[:p])
                nc.vector.reciprocal(rstd[:p], rstd[:p])
                nc.vector.tensor_scalar(
                    xn_bf16[:p], x_bf16[:p], mv[:p, 0:1], rstd[:p],
                    ALU.subtract, ALU.mult,
                )

            sloc = [dram.tile([NQ, H], F32, tag=f"sloc{b}", name=f"sloc{b}")
                    for b in range(B)]
            sglob = [dram.tile([NQ, H], F32, tag=f"sglob{b}", name=f"sglob{b}")
                     for b in range(B)]
            # normalized, transposed context for all batches: [p, b, k, q]
            ctxT_all = sb.tile([128, B, NDC, 112], BF16, tag="ctxTall")
            NQP = 112  # q rows padded to the XBAR 16-row granule
            qnT = sb.tile([128, B, NDC, NQP], BF16, tag="qnT")
            qpT = []

            def load_kv(b):
                kvraws = []
                for r in range(NKT):
                    kvraw = sb.tile([128, D], BF16, tag="kvraw", bufs=4)
                    nc.sync.dma_start(kvraw[:], kv_d[b, r])
                    kvraws.append(kvraw)
                return kvraws

            def transpose_kv(b, kvraws):
                """LayerNorm + XBAR DMA transpose into kvnT[b].

                kvnT layout: [128 dpart, NKT, NDC, 128 keys]."""
                kvnT = sb.tile([128, NKT, NDC, 128], BF16, tag="kvnT", bufs=3,
                               name=f"kvnT_{b}")
                for r in range(NKT):
                    layernorm_to_bf16(kvraws[r], kvraws[r], 128)
                    nc.sync.dma_start_transpose(kvnT[:, r], kvraws[r][:])
                return kvnT

            def kproj(b, kvnT):
                """K projection -> kpT[j]: [128 dout, KC keys] (no bias)."""
                kpT = []
                for j in range(NDC):
                    kpT.append(
                        sb.tile([128, KC], BF16, tag=f"kpT{j}", bufs=2,
                                name=f"kpT{j}_{b}")
                    )
                    acc = ps.tile([128, KC], F32, tag="mm", bufs=3)
                    for k in range(NDC):
                        nc.tensor.matmul(
                            acc[:],
                            lhsT=wk_sb[:, k, j * 128:(j + 1) * 128],
                            rhs=kvnT[:, :, k, :],
                            start=(k == 0), stop=(k == NDC - 1),
                        )
                    nc.scalar.activation(kpT[j][:], acc[:], AF.Copy)
                return kpT

            def vproj(b, kvnT):
                """V projection -> vp_ext[r]: [128 keys, H, HD+1], col HD=1."""
                vp_ext = []
                for r in range(NKT):
                    vpe = sb.tile([128, H, HD + 1], BF16, tag=f"vpe{r}", bufs=2,
                                  name=f"vpe{r}_{b}")
                    vp_ext.append(vpe)
                    nc.vector.memset(vpe[:, :, HD:HD + 1], 1.0)
                    for nh in range(2):
                        acc = ps.tile([128, 512], F32, tag="mm", bufs=3)
                        for k in range(NDC):
                            nc.tensor.matmul(
                                acc[:],
                                lhsT=kvnT[:, r, k, :],
                                rhs=wv_sb[:, k, nh * 512:(nh + 1) * 512],
                                start=(k == 0), stop=(k == NDC - 1),
                            )
                        if nh == 0:
                            nc.vector.tensor_copy(
                                out=vpe[:, nh * 8:(nh + 1) * 8, 0:HD],
                                in_=acc[:].rearrange("p (g d) -> p g d", g=8),
                            )
                        else:
                            nc.scalar.activation(
                                vpe[:, nh * 8:(nh + 1) * 8, 0:HD],
                                acc[:].rearrange("p (g d) -> p g d", g=8),
                                AF.Copy,
                            )
                return vp_ext

            def load_q():
                qraws = []
                for b in range(B):
                    qraw = sb.tile([NQ, D], BF16, tag="qraw", bufs=2)
                    nc.sync.dma_start(qraw[:], q_d[b])
                    qraws.append(qraw)
                return qraws

            def ln_T_q1(qraw, b):
                qn = sb.tile([NQP, D], BF16, tag="qn", bufs=2)
                layernorm_to_bf16(qraw, qn, NQ)
                nc.sync.dma_start_transpose(qnT[:, b], qn[:])

            for j in range(NDC):
                qpT.append(
                    sb.tile([128, B, 2, NQ], BF16, tag=f"qpT{j}",
                            name=f"qpT{j}")
                )

            def qproj(b0):
                """qpT_pad[j][:, b0:b0+2]: block-diagonal by head, for one
                batch pair (lets scores(0) start after only two q
                transposes)."""
                for j in range(NDC):
                    if b0 == 0:
                        nc.gpsimd.memset(qpT[j][:], 0.0)
                    acc = ps.tile([128, 2 * NQ], F32, tag="sc", bufs=2)
                    for k in range(NDC):
                        nc.tensor.matmul(
                            acc[:],
                            lhsT=wq_sb[:, k, j * 128:(j + 1) * 128],
                            rhs=qnT[:, b0:b0 + 2, k, 0:NQ],
                            start=(k == 0), stop=(k == NDC - 1),
                        )
                    nc.scalar.activation(
                        qpT[j][0:HD, b0:b0 + 2, 0, :],
                        acc[0:HD, :].rearrange("p (b q) -> p b q", b=2),
                        AF.Identity, bias=bq_sb[j][0:HD],
                    )
                    nc.scalar.activation(
                        qpT[j][HD:128, b0:b0 + 2, 1, :],
                        acc[HD:128, :].rearrange("p (b q) -> p b q", b=2),
                        AF.Identity, bias=bq_sb[j][HD:128],
                    )

            mask_all = sb.tile([128, B, NKT, 2, NQ], BF16, tag="maskb")

            def scores_exp(b, kpT):
                """scores^T + additive mask bias + exp for all heads of b."""
                exp_all = sb.tile([128, NKT, H, NQ], BF16, tag="expall", bufs=2,
                                  name=f"exp_{b}")
                for j in range(NDC):
                    for c2 in range(2):
                        sc = ps.tile([128, 2, 2, NQ], F32, tag="sc", bufs=2)
                        for cc in range(2):
                            c = 2 * c2 + cc
                            nc.tensor.matmul(
                                sc[:, cc, :, :].rearrange("p i q -> p (i q)"),
                                lhsT=kpT[j][:, c * 128:(c + 1) * 128],
                                rhs=qpT[j][:, b, :, :].rearrange(
                                    "p i q -> p (i q)"),
                                start=True, stop=True,
                            )
                        for cc in range(2):
                            c = 2 * c2 + cc
                            nc.vector.tensor_add(
                                sc[:, cc, :, :], sc[:, cc, :, :],
                                mask_all[:, b, c, :, :],
                            )
                        nc.scalar.activation(
                            exp_all[:, 2 * c2:2 * c2 + 2, 2 * j:2 * j + 2, :],
                            sc[:], AF.Exp, scale=float(SCALE),
                        )
                return exp_all

            def ctx_block(b, exp_all, vp_ext):
                """Unnormalized ctx + denominators for batch b; DMA the local
                denominators out for the all-reduce."""
                ctx_b = sb.tile([NQ, H, HD + 1], BF16, tag="ctxsb", bufs=4,
                                name=f"ctx_{b}")
                for h in range(H):
                    ctx_ps = ps.tile([NQ, HD + 1], F32, tag="small", bufs=3)
                    for c in range(NKT):
                        nc.tensor.matmul(
                            ctx_ps[:],
                            lhsT=exp_all[:, c, h, :],
                            rhs=vp_ext[c][:, h, :],
                            start=(c == 0), stop=(c == NKT - 1),
                        )
                    if h % 2 == 0:
                        nc.vector.tensor_copy(out=ctx_b[:, h, :], in_=ctx_ps[:])
                    else:
                        nc.scalar.activation(ctx_b[:, h, :], ctx_ps[:], AF.Copy)
                den = sb.tile([NQ, H], F32, tag="den", bufs=2)
                nc.gpsimd.tensor_copy(out=den[:], in_=ctx_b[:, :, HD])
                nc.sync.dma_start(sloc[b][:], den[:])
                return ctx_b

            def allreduce_b(b):
                nc.gpsimd.collective_compute(
                    "AllReduce", ALU.add,
                    replica_groups=[list(range(NCORE))],
                    ins=[sloc[b][:].opt()], outs=[sglob[b][:].opt()],
                )
                return b

            def recip_fetch(b):
                recip = sb.tile([NQ, H], F32, tag="recip", bufs=4,
                                name=f"recip_{b}")
                nc.scalar.dma_start(recip[:], sglob[b][:])
                return recip

            def combine_block(b, ctx_b, recip):
                """Normalize by global denominators and XBAR-transpose into
                ctxT_all."""
                nc.vector.reciprocal(recip[:], recip[:])
                ctxn = sb.tile([112, H, HD], BF16, tag="ctxn", bufs=1)
                for h in range(H):
                    if h % 2 == 0:
                        nc.vector.tensor_scalar_mul(
                            ctxn[:NQ, h, :], ctx_b[:, h, 0:HD],
                            recip[:, h:h + 1]
                        )
                    else:
                        nc.scalar.activation(
                            ctxn[:NQ, h, :], ctx_b[:, h, 0:HD], AF.Copy,
                            scale=recip[:, h:h + 1],
                        )
                nc.scalar.dma_start_transpose(ctxT_all[:, b], ctxn[:])

            # reuses wq's slot (wq is dead after the Q projection)
            out_sb = sb.tile([128, NDC, B, NQ], BF16, tag="wq", bufs=1,
                             name="out_sb")

            def outproj(b0, nb):
                for m in range(NDC):
                    acc = ps.tile([128, nb, NQ], F32, tag="sc", bufs=2)
                    for k in range(NDC):
                        nc.tensor.matmul(
                            acc[:],
                            lhsT=wo_sb[:, k, m * 128:(m + 1) * 128],
                            rhs=ctxT_all[:, b0:b0 + nb, k, 0:NQ],
                            start=(k == 0), stop=(k == NDC - 1),
                        )
                    if m % 2 == 0:
                        nc.vector.tensor_copy(
                            out=out_sb[:, m, b0:b0 + nb, :], in_=acc[:]
                        )
                    else:
                        nc.scalar.activation(
                            out_sb[:, m, b0:b0 + nb, :], acc[:], AF.Copy,
                        )
                nc.sync.dma_start(
                    out_d[:, :, b0:b0 + nb, :], out_sb[:, :, b0:b0 + nb, :]
                )

            # ---- pipelined schedule (v5) ----
            # Laws this schedule is built around (verified in traces):
            #  1. An XBAR transpose waits for every DMA emitted before it to
            #     COMPLETE (deadlock guard) and blocks its ring while
            #     waiting; q/kv transposes ride the sync ring interleaved
            #     with the loads in need-order.
            #  2. The ACT engine queue is strict FIFO: recip fetches are
            #     emitted after the following vproj's PSUM copies so a
            #     pending all-reduce never stalls exp/copy work.
            #  3. A collective waits for every previously emitted XBAR to
            #     complete; DMAs emitted after a collective wait for it.
            #  4. ctx(b) runs right after scores(b) so the denominator
            #     all-reduce fires as early as possible.
            qraws = load_q()
            nc.sync.dma_start(bqv_sb[:], bq_d[:])
            ln_T_q1(qraws[0], 0)
            ln_T_q1(qraws[1], 1)

            kvraws0 = load_kv(0)
            nc.sync.dma_start(wk_sb[:], wk_d[:])
            for j in range(NDC):
                nc.sync.dma_start(
                    wq_sb[:, :, j * 128:(j + 1) * 128],
                    wq_d[:, :, j * 128:(j + 1) * 128],
                )
            qproj(0)
            kvnT0 = transpose_kv(0, kvraws0)
            ln_T_q1(qraws[2], 2)
            ln_T_q1(qraws[3], 3)

            kpT0 = kproj(0, kvnT0)
            kvraws1 = load_kv(1)
            nc.sync.dma_start(wv_sb[:], wv_d[:])
            nc.sync.dma_start(mask_all[:], mask_d[:])
            qproj(2)
            kvnT1 = transpose_kv(1, kvraws1)

            vp0 = vproj(0, kvnT0)
            kvraws2 = load_kv(2)
            kvnT2 = transpose_kv(2, kvraws2)
            kvraws3 = load_kv(3)
            nc.sync.dma_start(wo_sb[:], wo_d[:])
            kvnT3 = transpose_kv(3, kvraws3)

            exp0 = scores_exp(0, kpT0)
            ctx0 = ctx_block(0, exp0, vp0)
            allreduce_b(0)
            kpT1 = kproj(1, kvnT1)
            vp1 = vproj(1, kvnT1)
            exp1 = scores_exp(1, kpT1)
            r0 = recip_fetch(0)
            combine_block(0, ctx0, r0)

            ctx1 = ctx_block(1, exp1, vp1)
            allreduce_b(1)
            kpT2 = kproj(2, kvnT2)
            vp2 = vproj(2, kvnT2)
            exp2 = scores_exp(2, kpT2)
            r1 = recip_fetch(1)
            combine_block(1, ctx1, r1)

            ctx2 = ctx_block(2, exp2, vp2)
            allreduce_b(2)
            kpT3 = kproj(3, kvnT3)
            vp3 = vproj(3, kvnT3)
            exp3 = scores_exp(3, kpT3)
            r2 = recip_fetch(2)
            combine_block(2, ctx2, r2)

            outproj(0, 2)
            ctx3 = ctx_block(3, exp3, vp3)
            allreduce_b(3)
            outproj(2, 1)
            r3 = recip_fetch(3)
            combine_block(3, ctx3, r3)
            outproj(3, 1)

    nc.compile()
    return nc


def _prep_in_maps(q, kv, mask, in_proj_w, in_proj_b, out_w, out_b,
                  g_q, b_q, g_kv, b_kv):
    """Host-side prep: fold LN affine + V-bias, shard kv/mask per core.

    Returns (in_maps, bias_total)."""
    q = np.asarray(q, np.float32)
    kv = np.asarray(kv, np.float32)
    mask = np.asarray(mask)
    in_proj_w = np.asarray(in_proj_w, np.float32)
    in_proj_b = np.asarray(in_proj_b, np.float32)
    out_w = np.asarray(out_w, np.float32)
    out_b = np.asarray(out_b, np.float32)
    g_q = np.asarray(g_q, np.float32)
    b_q = np.asarray(b_q, np.float32)
    g_kv = np.asarray(g_kv, np.float32)
    b_kv = np.asarray(b_kv, np.float32)

    Wq, Wk, Wv = in_proj_w[:D], in_proj_w[D:2 * D], in_proj_w[2 * D:]
    bq, bk, bv = in_proj_b[:D], in_proj_b[D:2 * D], in_proj_b[2 * D:]

    # Fold LayerNorm affine into projections: LN(x)*g+b @ W^T + c
    #   = LN(x) @ (W*g)^T + (W@b + c)
    WqT = (Wq * g_q[None, :]).T.astype(ml_dtypes.bfloat16)
    WkT = (Wk * g_kv[None, :]).T.astype(ml_dtypes.bfloat16)
    WvT = (Wv * g_kv[None, :]).T.astype(ml_dtypes.bfloat16)
    bq_eff = (bq + Wq @ b_q).astype(np.float32)
    bv_eff = (bv + Wv @ b_kv).astype(np.float32)
    # K bias cancels in softmax; V bias folds into the output bias.
    WoT = out_w.T.astype(ml_dtypes.bfloat16)
    bias_total = (out_b + out_w @ bv_eff).astype(np.float32)

    # per-query key mask; all-zero mask rows attend everywhere
    kv16 = kv.astype(ml_dtypes.bfloat16)
    allowed = (mask != 0)
    has_any = allowed.any(axis=-1, keepdims=True)
    eff = np.where(has_any, allowed, True)  # [B, NQ, HW] bool

    common = {
        "q": np.ascontiguousarray(q.astype(ml_dtypes.bfloat16)),
        "wqT": np.ascontiguousarray(WqT.reshape(NDC, 128, D).transpose(1, 0, 2)),
        "wkT": np.ascontiguousarray(WkT.reshape(NDC, 128, D).transpose(1, 0, 2)),
        "wvT": np.ascontiguousarray(WvT.reshape(NDC, 128, D).transpose(1, 0, 2)),
        "woT": np.ascontiguousarray(WoT.reshape(NDC, 128, D).transpose(1, 0, 2)),
        "biasq": np.ascontiguousarray(bq_eff.reshape(NDC, 128).T),
    }
    in_maps = []
    for c in range(NCORE):
        sl = slice(c * KC, (c + 1) * KC)
        kv_c = kv16[:, sl, :].reshape(B, NKT, 128, D)
        # additive mask bias slice -> [128, B, NKT, 2, NQ] bf16
        m_c = eff[:, :, sl].transpose(0, 2, 1).reshape(B, NKT, 128, NQ)
        mb = np.where(m_c, np.float32(0.0), np.float32(-1e9))
        mb = mb.transpose(2, 0, 1, 3)  # [128, B, NKT, NQ]
        mb = np.broadcast_to(mb[:, :, :, None, :], (128, B, NKT, 2, NQ))
        in_maps.append({
            **common,
            "kv": np.ascontiguousarray(kv_c),
            "maskT": np.ascontiguousarray(mb.astype(ml_dtypes.bfloat16)),
        })
    return in_maps, bias_total


def kernel(q, kv, mask, in_proj_w, in_proj_b, out_w, out_b, g_q, b_q, g_kv, b_kv):
    in_maps, bias_total = _prep_in_maps(
        q, kv, mask, in_proj_w, in_proj_b, out_w, out_b, g_q, b_q, g_kv, b_kv
    )
    if "nc" not in _compiled:
        _compiled["nc"] = _build()
    nc = _compiled["nc"]

    res = run_bass_kernel_spmd(nc, in_maps, core_ids=list(range(NCORE)))

    out = np.zeros((B, NQ, D), np.float32)
    for c in range(NCORE):
        part = res.results[c]["out"]  # [128 p, NDC m, B, NQ]; dout = m*128+p
        out += part.transpose(2, 3, 1, 0).reshape(B, NQ, D).astype(np.float32)
    out += bias_total[None, None, :]
    return out
